# revision 26
# baseline (speedup 1.0000x reference)
"""DiffGraphTransformer attention kernel for 8x Trainium2 NeuronCores.

Reference computation (T=1024, B=8, E=512, H=8, hd=64):
    qkv = query @ in_proj_weight.T + in_proj_bias ; q,k,v = split(qkv)
    k = q ; q *= hd**-0.5
    per (batch,head): scores = q @ k.T            (T,T)
                      w = exp(scores - max) * pe[b]
                      w /= clip(sum(w,-1), 1e-6)
                      attn = w @ v
    out = attn @ out_proj_weight.T + out_proj_bias

Sharding: batch b -> core b.  Heads 8b..8b+7 all use pe[b], so each core is
fully independent (pure SPMD, no collectives, full inputs sharded on host).

Algebraic restructuring (exact up to fp rounding):
  * k == q, so the k-chunk of in_proj is dead weight; only Wq / Wv used.
  * softmax max-subtraction replaced by a constant shift (exp(s/8 - 10)):
    cancels in the normalization, keeps exp() inside fp16 range.
  * S = q q^T is symmetric (bit-identical across the diagonal since both
    matmul operands read the same qT buffer).  E = exp(S) stored [s, t]
    times pe^T gives W'[s,t] = w[t,s] - exactly the contraction-major
    operand the attention matmul needs; no (T,T) transpose ever happens.
  * attention lhsT = [v_h | ones] (128, 65): row 64 of the PSUM output is
    the softmax denominator, for free.
  * v-bias and out-bias fold into bo2 = Wo @ bv + bo (host precomputed),
    added via a K=1 ones matmul in the out-proj accumulation.
  * the pair's 4 denominator rows are DMA-reshaped to (32, 64) so ONE
    reciprocal covers them at 64 elems/lane (DVE recip is free-dim bound),
    then scattered through DRAM and partition-broadcast for the
    normalization multiply applied in SBUF.

Engine assignment: PE does projections/out-proj in float32r (e8m11, 1
cyc/row) and scores/attention in fp16; ACT does exp (the ~1us/(128,1024)
pacer); DVE does PSUM evacuations + most pe-multiplies (fp16 2x mode);
GPSIMD takes the last 2 pe-multiplies of each head; DMA handles the
reciprocal reshape/broadcast.  P2 is software-pipelined: iteration j
computes pair j's scores/exp/W' while the attention matmuls consume pair
j-1 (its W' tiles are all ready, so the PE never waits on ACT), with the
first SKEW attention k-steps hoisted above the scores to pad the iteration
boundary.  The out-projection is split: k=0..2 partials overlap the last
attention iteration; only k=3 + bias + store remain at the end.

Measured on trn2 (8 cores, whole kernel): 184.6 us, rel err 3.9e-4.
Known remaining headroom: the PE HAM clock gate spends roughly half the
kernel at 1.2 GHz - a single N=512 matmul stream shows only ~50% activity
at K=4/8 so it cannot re-warm itself; only concurrently-issued row-packed
score pairs push activity over the threshold.  Deeper scores PSUM rings
would fix the exp-lag stalls that trigger re-throttles, but PSUM (8 banks)
is exactly full.  Exploiting E's symmetry (exp only the triangle + DMA
transpose for the mirror) could cut the ACT exp load ~2x if ever needed.
"""

import sys

for _p in ("/opt/trn_rl_repo",):
    if _p not in sys.path:
        sys.path.insert(0, _p)

import numpy as np
import ml_dtypes

T, B, E = 1024, 8, 512
H = 8
HD = E // H  # 64
N_CORES = 8

# ---- tunables -------------------------------------------------------------
MM_DT = "float32r"  # dtype for PE matmuls: "float32r" | "float32" | "bfloat16"
W_DT = "float16"   # dtype of E / W' / v (the attention operands)
GPSIMD_MUL_TILES = 2  # of the 8 s-tiles of each head's W' multiply, how many go to GPSIMD
# ---------------------------------------------------------------------------

# global constant subtracted inside exp (cancels in normalization; keeps
# exp() outputs inside fp16 range: scores/8 - 10 is in [-16, ~6])
EXP_SHIFT = -10.0

_cache = {}


def _build_nc():
    import concourse.bass as bass
    import concourse.tile as tile
    import concourse.mybir as mybir
    from concourse import bacc
    from contextlib import ExitStack

    f32 = mybir.dt.float32
    bf16 = mybir.dt.bfloat16
    mm_dt = getattr(mybir.dt, MM_DT)
    w_dt = getattr(mybir.dt, W_DT)
    Exp = mybir.ActivationFunctionType.Exp

    nc = bacc.Bacc("TRN2", debug=False)

    # DRAM I/O (per-core contents supplied via in_maps)
    xT_d = nc.dram_tensor("xT", [E, T], mm_dt, kind="ExternalInput").ap()
    peT_d = nc.dram_tensor("peT", [T, T], w_dt, kind="ExternalInput").ap()
    wqT_d = nc.dram_tensor("wqT", [E, E], mm_dt, kind="ExternalInput").ap()
    wvT_d = nc.dram_tensor("wvT", [E, E], mm_dt, kind="ExternalInput").ap()
    woT_d = nc.dram_tensor("woT", [E, E], mm_dt, kind="ExternalInput").ap()
    bq_d = nc.dram_tensor("bq", [E], f32, kind="ExternalInput").ap()
    bo2_d = nc.dram_tensor("bo2", [E], mm_dt, kind="ExternalInput").ap()
    ones_d = nc.dram_tensor("ones1", [128], mm_dt, kind="ExternalInput").ap()
    out_d = nc.dram_tensor("out", [T, E], f32, kind="ExternalOutput").ap()

    KT = E // 128   # 4 contraction tiles for the projections
    TT = T // 128   # 8 t-tiles
    NH = T // 512   # 2 psum-bank halves of the t dimension

    def mm(ap):
        return ap

    with ExitStack() as ctx:
        tc = ctx.enter_context(tile.TileContext(nc))

        sing = ctx.enter_context(tc.tile_pool(name="sing", bufs=1))
        p_in = ctx.enter_context(tc.tile_pool(name="p_in", bufs=1))
        p_qv = ctx.enter_context(tc.tile_pool(name="p_qv", bufs=1))
        p_E = ctx.enter_context(tc.tile_pool(name="p_E", bufs=10))
        p_W = ctx.enter_context(tc.tile_pool(name="p_W", bufs=20))
        p_rc = ctx.enter_context(tc.tile_pool(name="p_rc", bufs=4))
        p_rm = ctx.enter_context(tc.tile_pool(name="p_rm", bufs=2))
        p_st = ctx.enter_context(tc.tile_pool(name="p_st", bufs=2))
        p_dr = ctx.enter_context(tc.tile_pool(name="p_dr", bufs=4, space="DRAM"))
        ps_a = ctx.enter_context(tc.tile_pool(name="ps_a", bufs=2, space="PSUM"))
        ps_b = ctx.enter_context(tc.tile_pool(name="ps_b", bufs=4, space="PSUM"))

        # ---- constants / weights into SBUF --------------------------------
        wq_sb = [sing.tile([128, E], mm_dt, tag=f"wq{k}", name="wq") for k in range(KT)]
        wv_sb = [sing.tile([128, E], mm_dt, tag=f"wv{k}", name="wv") for k in range(KT)]
        wo_sb = [sing.tile([128, E], mm_dt, tag=f"wo{k}", name="wo") for k in range(KT)]
        bq_sb = [sing.tile([128, 1], f32, tag=f"bq{k}", name="bq") for k in range(KT)]
        for k in range(KT):
            nc.sync.dma_start(out=wq_sb[k], in_=wqT_d[k * 128:(k + 1) * 128, :])
            nc.sync.dma_start(out=wv_sb[k], in_=wvT_d[k * 128:(k + 1) * 128, :])
            nc.sync.dma_start(out=wo_sb[k], in_=woT_d[k * 128:(k + 1) * 128, :])
            nc.sync.dma_start(out=bq_sb[k], in_=bq_d[k * 128:(k + 1) * 128].rearrange("(p one) -> p one", one=1))
        ebias = sing.tile([128, 1], f32, tag="ebias")
        nc.vector.memset(ebias, EXP_SHIFT)
        ones1 = sing.tile([1, 128], mm_dt, tag="ones1")
        nc.sync.dma_start(out=ones1, in_=ones_d.unsqueeze(0))
        bo2_sb = sing.tile([1, E], mm_dt, tag="bo2")
        nc.sync.dma_start(out=bo2_sb, in_=bo2_d.unsqueeze(0))

        xT_sb = [p_in.tile([128, T], mm_dt, tag=f"xT{k}", name="xT") for k in range(KT)]
        for k in range(KT):
            nc.sync.dma_start(out=xT_sb[k], in_=xT_d[k * 128:(k + 1) * 128, :])

        peT_sb = [p_in.tile([128, T], w_dt, tag=f"peT{k}", name="peT") for k in range(TT)]
        for i in range(TT):
            nc.sync.dma_start(out=peT_sb[i], in_=peT_d[i * 128:(i + 1) * 128, :])

        # ---- P1: projections ----------------------------------------------
        # qT[e_out, t] with e_out on partitions (4 tiles); includes q-bias.
        qT_sb = [p_qv.tile([128, T], w_dt, tag=f"qT{k}", name="qT") for k in range(KT)]
        # v[t, e_out] natural, with a ones column appended per head:
        # layout (128, 8*65): head h occupies cols [65h, 65h+64), ones at 65h+64.
        v_sb = [p_qv.tile([128, H * (HD + 1)], w_dt, tag=f"v{k}", name="v") for k in range(TT)]

        psum_ring = []
        for m in range(KT):          # qT projection
            for nh in range(NH):
                ps = (ps_a if (m * NH + nh) % 2 == 0 else ps_b).tile(
                    [128, 512], f32, tag="slot", name="pp")
                for k in range(KT):
                    nc.tensor.matmul(
                        ps, mm(wq_sb[k][:, m * 128:(m + 1) * 128]),
                        mm(xT_sb[k][:, nh * 512:(nh + 1) * 512]),
                        start=(k == 0), stop=(k == KT - 1))
                nc.vector.tensor_scalar_add(
                    qT_sb[m][:, nh * 512:(nh + 1) * 512], ps, bq_sb[m])

        for mt in range(TT):         # v projection
            ps = (ps_a if mt % 2 == 0 else ps_b).tile([128, 512], f32, tag="slot", name="pp")
            for k in range(KT):
                nc.tensor.matmul(
                    ps, mm(xT_sb[k][:, mt * 128:(mt + 1) * 128]), mm(wv_sb[k]),
                    start=(k == 0), stop=(k == KT - 1))
            v_dst = v_sb[mt].rearrange("p (h c) -> p h c", c=HD + 1)
            nc.vector.tensor_copy(
                v_dst[:, :, 0:HD],
                ps.rearrange("p (h c) -> p h c", c=HD))
            nc.vector.memset(v_dst[:, :, HD:HD + 1], 1.0)

        # ---- P2: attention, software-pipelined over head pairs ------------
        # Iteration j produces W' for pair j (scores -> exp -> pe-multiply)
        # while the attention matmuls consume pair j-1's W' tiles.  This
        # keeps the PE stream dense across pair boundaries (HAM stays warm).
        attnT_sb = [p_qv.tile([128, T], mm_dt, tag=f"attnT{k}", name="attnT") for k in range(KT)]
        NP = H // 2  # pairs
        Ws_of = {}   # pair j -> [hh][i] W' tiles

        def emit_scores(j, i):
            # nh-major emission: consecutive MMs alternate row groups
            # (head A rows 0-63, head B rows 64-127) so the PE runs the
            # pair concurrently and overlaps B's LDWEIGHTS with A's MM.
            qt = qT_sb[j]
            scs = [ps_a.tile([128, T], f32, tag="slot", name="sc") for _ in range(2)]
            for nh in range(NH):
                for hh in range(2):
                    r0 = hh * HD
                    nc.tensor.matmul(
                        scs[hh][:, nh * 512:(nh + 1) * 512],
                        mm(qt[r0:r0 + HD, i * 128:(i + 1) * 128]),
                        mm(qt[r0:r0 + HD, nh * 512:(nh + 1) * 512]),
                        start=True, stop=True,
                        tile_position=(r0, 0))
            for hh in range(2):
                Et = p_E.tile([128, T], w_dt, tag="E", name="Et")
                nc.scalar.activation(out=Et, in_=scs[hh], func=Exp, scale=0.125, bias=ebias)
                Wt = p_W.tile([128, T], w_dt, tag="W", name="Wt")
                if i >= TT - GPSIMD_MUL_TILES:
                    nc.gpsimd.tensor_mul(Wt, Et, peT_sb[i])
                else:
                    nc.vector.tensor_mul(Wt, Et, peT_sb[i])
                Ws_of[j][hh][i] = Wt

        def emit_attn_kstep(j, i, at_ps):
            for hh in range(2):
                vcol = 65 * (2 * j + hh)
                for nh in range(NH):
                    nc.tensor.matmul(
                        at_ps[hh][nh],
                        v_sb[i][:, vcol:vcol + HD + 1],
                        Ws_of[j][hh][i][:, nh * 512:(nh + 1) * 512],
                        start=(i == 0), stop=(i == TT - 1))

        def emit_evac(j, at_ps):
            # plain evacuation first (frees the attention PSUM slots fast);
            # normalization happens afterwards in SBUF, in place.
            for hh in range(2):
                for nh in range(NH):
                    nc.vector.tensor_copy(
                        attnT_sb[j][hh * HD:(hh + 1) * HD,
                                    nh * 512:(nh + 1) * 512],
                        at_ps[hh][nh][0:HD, :])
            rows = p_rc.tile([HD + 1, 4 * 512], f32, tag="rc", name="rows", bufs=2)
            for hh in range(2):
                for nh in range(NH):
                    r = hh * NH + nh
                    nc.vector.tensor_copy(
                        rows[HD:HD + 1, r * 512:(r + 1) * 512],
                        at_ps[hh][nh][HD:HD + 1, :])
            # one low-free-dim reciprocal for the pair's 4 denominator rows
            rg = p_rc.tile([32, 64], f32, tag="rg", name="rg")
            nc.sync.dma_start(
                out=rg,
                in_=rows[HD:HD + 1, :].rearrange("one (a c) -> one a c", c=64))
            rgi = p_rc.tile([32, 64], f32, tag="rgi", name="rgi")
            nc.vector.reciprocal(rgi, rg)
            rrow = p_dr.tile([1, 4 * 512], f32, tag="rrow", name="rrow")
            nc.sync.dma_start(
                out=rrow.rearrange("one (a c) -> one a c", c=64), in_=rgi)
            for hh in range(2):
                for nh in range(NH):
                    r = hh * NH + nh
                    rm = p_rm.tile([128, 512], f32, tag="rm", name="rm")
                    rms = rm[hh * HD:(hh + 1) * HD, :]
                    seg = rrow[0:1, r * 512:(r + 1) * 512]
                    bcast = bass.AP(tensor=seg.tensor, offset=seg.offset,
                                    ap=[[0, HD]] + list(seg.ap[1:]))
                    nc.sync.dma_start(out=rms, in_=bcast)
                    dst = attnT_sb[j][hh * HD:(hh + 1) * HD,
                                      nh * 512:(nh + 1) * 512]
                    nc.vector.tensor_mul(dst, dst, rms)

        p3_part = [p_qv.tile([128, E], f32, tag=f"p3p{k}", name="p3p")
                   for k in range(TT)]

        def emit_p3a(mt):
            # out-projection partial over k=0..2 (heads 0..5); evacuated to
            # SBUF so the PSUM slot recycles while pair 3 finishes.
            ps = ps_a.tile([128, 512], f32, tag="slot", name="pp")
            for k in range(KT - 1):
                nc.tensor.matmul(
                    ps, mm(attnT_sb[k][:, mt * 128:(mt + 1) * 128]),
                    mm(wo_sb[k]), start=(k == 0), stop=(k == KT - 2))
            nc.vector.tensor_copy(p3_part[mt], ps)

        # Skewed schedule: at each iteration boundary the first 3 attention
        # k-steps (which need no ACT results) run before the first scores
        # matmul, absorbing the exp-lag so the PE stream never gaps long
        # enough for the HAM clock gate to re-throttle.
        SKEW = 3
        for j in range(NP + 1):
            if j < NP:
                Ws_of[j] = [[None] * TT, [None] * TT]
                if j >= 2:
                    del Ws_of[j - 2]
            cur_at = None
            if j >= 1:
                cur_at = [[ps_b.tile([HD + 1, 512], f32, tag="slot", name="at")
                           for _ in range(NH)] for _ in range(2)]
                for i in range(SKEW):
                    emit_attn_kstep(j - 1, i, cur_at)
            for i in range(TT):
                if j < NP:
                    emit_scores(j, i)
                if j >= 1 and i + SKEW < TT:
                    emit_attn_kstep(j - 1, i + SKEW, cur_at)
                if j == NP and i < TT - SKEW:
                    # final iteration: overlap the out-projection k=0..2
                    # partial accumulation (pairs 0..2 are normalized)
                    emit_p3a(i)
            if j == NP:
                for mt in range(TT - SKEW, TT):
                    emit_p3a(mt)
            if j >= 1:
                emit_evac(j - 1, cur_at)

        # ---- P3 tail: k=3 + bias, add the partial, store ------------------
        for mt in range(TT):
            ps = ps_a.tile([128, 512], f32, tag="slot", name="pp")
            nc.tensor.matmul(
                ps, mm(attnT_sb[KT - 1][:, mt * 128:(mt + 1) * 128]),
                mm(wo_sb[KT - 1]), start=True, stop=False)
            nc.tensor.matmul(ps, ones1, bo2_sb, start=False, stop=True)
            st = p_st.tile([128, E], f32, tag="st", name="st")
            nc.vector.tensor_add(st, ps, p3_part[mt])
            nc.sync.dma_start(out=out_d[mt * 128:(mt + 1) * 128, :], in_=st)

    nc.compile()
    return nc


def get_nc():
    if "nc" not in _cache:
        _cache["nc"] = _build_nc()
    return _cache["nc"]


def prep_inputs(query, pe, in_proj_weight, in_proj_bias, out_proj_weight,
                out_proj_bias):
    """Host-side sharding/layout prep. Returns per-core input maps."""
    query = np.asarray(query, dtype=np.float32)
    pe = np.asarray(pe, dtype=np.float32)
    in_proj_weight = np.asarray(in_proj_weight, dtype=np.float32)
    in_proj_bias = np.asarray(in_proj_bias, dtype=np.float32)
    out_proj_weight = np.asarray(out_proj_weight, dtype=np.float32)
    out_proj_bias = np.asarray(out_proj_bias, dtype=np.float32)

    def r32(x):
        # round-to-nearest-even onto e8m11 (fp32r keeps the top 20 bits)
        if MM_DT != "float32r":
            return np.ascontiguousarray(x, dtype=np.float32)
        u = np.ascontiguousarray(x, dtype=np.float32).view(np.uint32)
        u = (u + 0x7FF + ((u >> 12) & 1)) & np.uint32(0xFFFFF000)
        return u.view(np.float32)

    wqT = r32(in_proj_weight[0:E].T)                           # (E, E)
    wvT = r32(in_proj_weight[2 * E:3 * E].T)                   # (E, E)
    woT = r32(out_proj_weight.T)                               # (E, E)
    bq = np.ascontiguousarray(in_proj_bias[0:E])
    bv = in_proj_bias[2 * E:3 * E]
    bo2 = r32(out_proj_weight @ bv + out_proj_bias)

    in_maps = []
    for b in range(N_CORES):
        xT = r32(query[:, b, :].T)                             # (E, T)
        peT = np.ascontiguousarray(pe[b].T).astype(np.float16 if W_DT == "float16" else ml_dtypes.bfloat16)
        in_maps.append({
            "xT": xT, "peT": peT, "wqT": wqT, "wvT": wvT, "woT": woT,
            "bq": bq, "bo2": bo2, "ones1": np.ones(128, dtype=np.float32),
        })
    return in_maps


def kernel(query, pe, in_proj_weight, in_proj_bias, out_proj_weight,
           out_proj_bias):
    from concourse.bass_utils import run_bass_kernel_spmd

    nc = get_nc()
    in_maps = prep_inputs(query, pe, in_proj_weight, in_proj_bias,
                          out_proj_weight, out_proj_bias)
    res = run_bass_kernel_spmd(nc, in_maps, list(range(N_CORES)))
    out = np.empty((T, B, E), dtype=np.float32)
    for b in range(N_CORES):
        out[:, b, :] = res.results[b]["out"]
    return out


# revision 27
# speedup vs baseline: 1.1490x; 1.1490x over previous
"""DiffGraphTransformer attention kernel for 8x Trainium2 NeuronCores.

Reference computation (T=1024, B=8, E=512, H=8, hd=64):
    qkv = query @ in_proj_weight.T + in_proj_bias ; q,k,v = split(qkv)
    k = q ; q *= hd**-0.5
    per (batch,head): scores = q @ k.T            (T,T)
                      w = exp(scores - max) * pe[b]
                      w /= clip(sum(w,-1), 1e-6)
                      attn = w @ v
    out = attn @ out_proj_weight.T + out_proj_bias

Sharding: batch b -> core b.  Heads 8b..8b+7 all use pe[b], so each core is
fully independent (pure SPMD, no collectives, full inputs sharded on host).

Algebraic restructuring (exact up to fp rounding):
  * k == q, so the k-chunk of in_proj is dead weight; only Wq / Wv used.
  * softmax max-subtraction replaced by a constant shift (exp(s/8 - 10)):
    cancels in the normalization, keeps exp() inside fp16 range.
  * S = q q^T is symmetric (bit-identical across the diagonal since both
    matmul operands read the same qT buffer).  E = exp(S) stored [s, t]
    times pe^T gives W'[s,t] = w[t,s] - exactly the contraction-major
    operand the attention matmul needs; no (T,T) transpose ever happens.
  * attention lhsT = [v_h | ones] (128, 65): row 64 of the PSUM output is
    the softmax denominator, for free.
  * v-bias and out-bias fold into bo2 = Wo @ bv + bo (host precomputed),
    added via a K=1 ones matmul in the out-proj accumulation.
  * the pair's 4 denominator rows are DMA-reshaped to (32, 64) so ONE
    reciprocal covers them at 64 elems/lane (DVE recip is free-dim bound),
    then scattered through DRAM and partition-broadcast for the
    normalization multiply applied in SBUF.

Engine assignment: PE does projections/out-proj in float32r (e8m11, 1
cyc/row) and scores/attention in fp16; ACT does exp (the ~1us/(128,1024)
pacer); DVE does PSUM evacuations + most pe-multiplies (fp16 2x mode);
GPSIMD takes the last 2 pe-multiplies of each head; DMA handles the
reciprocal reshape/broadcast.  P2 is software-pipelined: iteration j
computes pair j's scores/exp/W' while the attention matmuls consume pair
j-1 (its W' tiles are all ready, so the PE never waits on ACT), with the
first SKEW attention k-steps hoisted above the scores to pad the iteration
boundary.  The out-projection is split: k=0..2 partials overlap the last
attention iteration; only k=3 + bias + store remain at the end.

Measured on trn2 (8 cores, whole kernel): 184.6 us, rel err 3.9e-4.
Known remaining headroom: the PE HAM clock gate spends roughly half the
kernel at 1.2 GHz - a single N=512 matmul stream shows only ~50% activity
at K=4/8 so it cannot re-warm itself; only concurrently-issued row-packed
score pairs push activity over the threshold.  Deeper scores PSUM rings
would fix the exp-lag stalls that trigger re-throttles, but PSUM (8 banks)
is exactly full.  Exploiting E's symmetry (exp only the triangle + DMA
transpose for the mirror) could cut the ACT exp load ~2x if ever needed.
"""

import sys

for _p in ("/opt/trn_rl_repo",):
    if _p not in sys.path:
        sys.path.insert(0, _p)

import numpy as np
import ml_dtypes

T, B, E = 1024, 8, 512
H = 8
HD = E // H  # 64
N_CORES = 8

# ---- tunables -------------------------------------------------------------
MM_DT = "float32r"  # dtype for PE matmuls: "float32r" | "float32" | "bfloat16"
W_DT = "float16"   # dtype of E / W' / v (the attention operands)
GPSIMD_MUL_TILES = 2  # of the 8 s-tiles of each head's W' multiply, how many go to GPSIMD
# ---------------------------------------------------------------------------

# global constant subtracted inside exp (cancels in normalization; keeps
# exp() outputs inside fp16 range: scores/8 - 10 is in [-16, ~6])
EXP_SHIFT = -10.0

_cache = {}


def _build_nc():
    import concourse.bass as bass
    import concourse.tile as tile
    import concourse.mybir as mybir
    from concourse import bacc
    from contextlib import ExitStack

    f32 = mybir.dt.float32
    bf16 = mybir.dt.bfloat16
    mm_dt = getattr(mybir.dt, MM_DT)
    w_dt = getattr(mybir.dt, W_DT)
    Exp = mybir.ActivationFunctionType.Exp

    nc = bacc.Bacc("TRN2", debug=False)

    # DRAM I/O (per-core contents supplied via in_maps)
    xT_d = nc.dram_tensor("xT", [E, T], mm_dt, kind="ExternalInput").ap()
    peT_d = nc.dram_tensor("peT", [T, T], w_dt, kind="ExternalInput").ap()
    wqT_d = nc.dram_tensor("wqT", [E, E], mm_dt, kind="ExternalInput").ap()
    wvT_d = nc.dram_tensor("wvT", [E, E], mm_dt, kind="ExternalInput").ap()
    woT_d = nc.dram_tensor("woT", [E, E], mm_dt, kind="ExternalInput").ap()
    bq_d = nc.dram_tensor("bq", [E], f32, kind="ExternalInput").ap()
    bo2_d = nc.dram_tensor("bo2", [E], mm_dt, kind="ExternalInput").ap()
    ones_d = nc.dram_tensor("ones1", [128], mm_dt, kind="ExternalInput").ap()
    out_d = nc.dram_tensor("out", [T, E], f32, kind="ExternalOutput").ap()

    KT = E // 128   # 4 contraction tiles for the projections
    TT = T // 128   # 8 t-tiles
    NH = T // 512   # 2 psum-bank halves of the t dimension

    def mm(ap):
        return ap

    with ExitStack() as ctx:
        tc = ctx.enter_context(tile.TileContext(nc))

        sing = ctx.enter_context(tc.tile_pool(name="sing", bufs=1))
        p_in = ctx.enter_context(tc.tile_pool(name="p_in", bufs=1))
        p_qv = ctx.enter_context(tc.tile_pool(name="p_qv", bufs=1))
        p_E = ctx.enter_context(tc.tile_pool(name="p_E", bufs=10))
        p_W = ctx.enter_context(tc.tile_pool(name="p_W", bufs=20))
        p_rc = ctx.enter_context(tc.tile_pool(name="p_rc", bufs=4))
        p_rm = ctx.enter_context(tc.tile_pool(name="p_rm", bufs=2))
        p_st = ctx.enter_context(tc.tile_pool(name="p_st", bufs=2))
        p_dr = ctx.enter_context(tc.tile_pool(name="p_dr", bufs=4, space="DRAM"))
        ps_a = ctx.enter_context(tc.tile_pool(name="ps_a", bufs=2, space="PSUM"))
        ps_b = ctx.enter_context(tc.tile_pool(name="ps_b", bufs=4, space="PSUM"))

        # ---- constants / weights into SBUF --------------------------------
        wq_sb = [sing.tile([128, E], mm_dt, tag=f"wq{k}", name="wq") for k in range(KT)]
        wv_sb = [sing.tile([128, E], mm_dt, tag=f"wv{k}", name="wv") for k in range(KT)]
        wo_sb = [sing.tile([128, E], mm_dt, tag=f"wo{k}", name="wo") for k in range(KT)]
        bq_sb = [sing.tile([128, 1], f32, tag=f"bq{k}", name="bq") for k in range(KT)]
        for k in range(KT):
            nc.sync.dma_start(out=wq_sb[k], in_=wqT_d[k * 128:(k + 1) * 128, :])
            nc.sync.dma_start(out=wv_sb[k], in_=wvT_d[k * 128:(k + 1) * 128, :])
            nc.sync.dma_start(out=wo_sb[k], in_=woT_d[k * 128:(k + 1) * 128, :])
            nc.sync.dma_start(out=bq_sb[k], in_=bq_d[k * 128:(k + 1) * 128].rearrange("(p one) -> p one", one=1))
        ebias = sing.tile([128, 1], f32, tag="ebias")
        nc.vector.memset(ebias, EXP_SHIFT)
        ones1 = sing.tile([1, 128], mm_dt, tag="ones1")
        nc.sync.dma_start(out=ones1, in_=ones_d.unsqueeze(0))
        bo2_sb = sing.tile([1, E], mm_dt, tag="bo2")
        nc.sync.dma_start(out=bo2_sb, in_=bo2_d.unsqueeze(0))

        xT_sb = [p_in.tile([128, T], mm_dt, tag=f"xT{k}", name="xT") for k in range(KT)]
        for k in range(KT):
            nc.sync.dma_start(out=xT_sb[k], in_=xT_d[k * 128:(k + 1) * 128, :])

        peT_sb = [p_in.tile([128, T], w_dt, tag=f"peT{k}", name="peT") for k in range(TT)]
        for i in range(TT):
            nc.sync.dma_start(out=peT_sb[i], in_=peT_d[i * 128:(i + 1) * 128, :])

        # ---- P1: projections ----------------------------------------------
        # qT[e_out, t] with e_out on partitions (4 tiles); includes q-bias.
        qT_sb = [p_qv.tile([128, T], w_dt, tag=f"qT{k}", name="qT") for k in range(KT)]
        # v[t, e_out] natural, with a ones column appended per head:
        # layout (128, 8*65): head h occupies cols [65h, 65h+64), ones at 65h+64.
        v_sb = [p_qv.tile([128, H * (HD + 1)], w_dt, tag=f"v{k}", name="v") for k in range(TT)]

        def emit_qproj(m, nh):
            ps = ps_b.tile([128, 512], f32, tag="slot", name="pp")
            for k in range(KT):
                nc.tensor.matmul(
                    ps, mm(wq_sb[k][:, m * 128:(m + 1) * 128]),
                    mm(xT_sb[k][:, nh * 512:(nh + 1) * 512]),
                    start=(k == 0), stop=(k == KT - 1))
            nc.vector.tensor_scalar_add(
                qT_sb[m][:, nh * 512:(nh + 1) * 512], ps, bq_sb[m])

        def emit_vproj(mt):
            ps = ps_b.tile([128, 512], f32, tag="slot", name="pp")
            for k in range(KT):
                nc.tensor.matmul(
                    ps, mm(xT_sb[k][:, mt * 128:(mt + 1) * 128]), mm(wv_sb[k]),
                    start=(k == 0), stop=(k == KT - 1))
            v_dst = v_sb[mt].rearrange("p (h c) -> p h c", c=HD + 1)
            nc.vector.tensor_copy(
                v_dst[:, :, 0:HD],
                ps.rearrange("p (h c) -> p h c", c=HD))
            nc.vector.memset(v_dst[:, :, HD:HD + 1], 1.0)

        # pair 0's qT upfront; the other 14 projection groups interleave
        # with iteration 0's scores so ACT starts exp'ing ~20us earlier and
        # the packable fp16 score pairs warm the PE clock gate early.
        for nh in range(NH):
            emit_qproj(0, nh)
        proj_rest = [("q", m, nh) for m in range(1, KT) for nh in range(NH)]
        proj_rest += [("v", mt, None) for mt in range(TT)]

        # ---- P2: attention, software-pipelined over head pairs ------------
        # Iteration j produces W' for pair j (scores -> exp -> pe-multiply)
        # while the attention matmuls consume pair j-1's W' tiles.  This
        # keeps the PE stream dense across pair boundaries (HAM stays warm).
        attnT_sb = [p_qv.tile([128, T], mm_dt, tag=f"attnT{k}", name="attnT") for k in range(KT)]
        NP = H // 2  # pairs
        Ws_of = {}   # pair j -> [hh][i] W' tiles

        def emit_scores(j, i):
            # nh-major emission: consecutive MMs alternate row groups
            # (head A rows 0-63, head B rows 64-127) so the PE runs the
            # pair concurrently and overlaps B's LDWEIGHTS with A's MM.
            qt = qT_sb[j]
            scs = [ps_a.tile([128, T], f32, tag="slot", name="sc") for _ in range(2)]
            for nh in range(NH):
                for hh in range(2):
                    r0 = hh * HD
                    nc.tensor.matmul(
                        scs[hh][:, nh * 512:(nh + 1) * 512],
                        mm(qt[r0:r0 + HD, i * 128:(i + 1) * 128]),
                        mm(qt[r0:r0 + HD, nh * 512:(nh + 1) * 512]),
                        start=True, stop=True,
                        tile_position=(r0, 0))
            for hh in range(2):
                Et = p_E.tile([128, T], w_dt, tag="E", name="Et")
                nc.scalar.activation(out=Et, in_=scs[hh], func=Exp, scale=0.125, bias=ebias)
                Wt = p_W.tile([128, T], w_dt, tag="W", name="Wt")
                if i >= TT - GPSIMD_MUL_TILES:
                    nc.gpsimd.tensor_mul(Wt, Et, peT_sb[i])
                else:
                    nc.vector.tensor_mul(Wt, Et, peT_sb[i])
                Ws_of[j][hh][i] = Wt

        def emit_attn_kstep(j, i, at_ps):
            for hh in range(2):
                vcol = 65 * (2 * j + hh)
                for nh in range(NH):
                    nc.tensor.matmul(
                        at_ps[hh][nh],
                        v_sb[i][:, vcol:vcol + HD + 1],
                        Ws_of[j][hh][i][:, nh * 512:(nh + 1) * 512],
                        start=(i == 0), stop=(i == TT - 1))

        def emit_evac(j, at_ps):
            # plain evacuation first (frees the attention PSUM slots fast);
            # normalization happens afterwards in SBUF, in place.
            for hh in range(2):
                for nh in range(NH):
                    nc.vector.tensor_copy(
                        attnT_sb[j][hh * HD:(hh + 1) * HD,
                                    nh * 512:(nh + 1) * 512],
                        at_ps[hh][nh][0:HD, :])
            rows = p_rc.tile([HD + 1, 4 * 512], f32, tag="rc", name="rows", bufs=2)
            for hh in range(2):
                for nh in range(NH):
                    r = hh * NH + nh
                    nc.vector.tensor_copy(
                        rows[HD:HD + 1, r * 512:(r + 1) * 512],
                        at_ps[hh][nh][HD:HD + 1, :])
            # one low-free-dim reciprocal for the pair's 4 denominator rows
            rg = p_rc.tile([32, 64], f32, tag="rg", name="rg")
            nc.sync.dma_start(
                out=rg,
                in_=rows[HD:HD + 1, :].rearrange("one (a c) -> one a c", c=64))
            rgi = p_rc.tile([32, 64], f32, tag="rgi", name="rgi")
            nc.vector.reciprocal(rgi, rg)
            rrow = p_dr.tile([1, 4 * 512], f32, tag="rrow", name="rrow")
            nc.sync.dma_start(
                out=rrow.rearrange("one (a c) -> one a c", c=64), in_=rgi)
            for hh in range(2):
                for nh in range(NH):
                    r = hh * NH + nh
                    rm = p_rm.tile([128, 512], f32, tag="rm", name="rm")
                    rms = rm[hh * HD:(hh + 1) * HD, :]
                    seg = rrow[0:1, r * 512:(r + 1) * 512]
                    bcast = bass.AP(tensor=seg.tensor, offset=seg.offset,
                                    ap=[[0, HD]] + list(seg.ap[1:]))
                    nc.sync.dma_start(out=rms, in_=bcast)
                    dst = attnT_sb[j][hh * HD:(hh + 1) * HD,
                                      nh * 512:(nh + 1) * 512]
                    nc.vector.tensor_mul(dst, dst, rms)

        p3_part = [p_qv.tile([128, E], f32, tag=f"p3p{k}", name="p3p")
                   for k in range(TT)]

        def emit_p3a(mt):
            # out-projection partial over k=0..2 (heads 0..5); evacuated to
            # SBUF so the PSUM slot recycles while pair 3 finishes.
            ps = ps_a.tile([128, 512], f32, tag="slot", name="pp")
            for k in range(KT - 1):
                nc.tensor.matmul(
                    ps, mm(attnT_sb[k][:, mt * 128:(mt + 1) * 128]),
                    mm(wo_sb[k]), start=(k == 0), stop=(k == KT - 2))
            nc.vector.tensor_copy(p3_part[mt], ps)

        # Skewed schedule: at each iteration boundary the first 3 attention
        # k-steps (which need no ACT results) run before the first scores
        # matmul, absorbing the exp-lag so the PE stream never gaps long
        # enough for the HAM clock gate to re-throttle.
        SKEW = 3
        for j in range(NP + 1):
            if j < NP:
                Ws_of[j] = [[None] * TT, [None] * TT]
                if j >= 2:
                    del Ws_of[j - 2]
            cur_at = None
            if j >= 1:
                cur_at = [[ps_b.tile([HD + 1, 512], f32, tag="slot", name="at")
                           for _ in range(NH)] for _ in range(2)]
                for i in range(SKEW):
                    emit_attn_kstep(j - 1, i, cur_at)
            for i in range(TT):
                if j < NP:
                    emit_scores(j, i)
                if j == 0:
                    for _ in range(2):
                        if proj_rest:
                            kind, a1, a2 = proj_rest.pop(0)
                            if kind == "q":
                                emit_qproj(a1, a2)
                            else:
                                emit_vproj(a1)
                if j >= 1 and i + SKEW < TT:
                    emit_attn_kstep(j - 1, i + SKEW, cur_at)
                if j == NP and i < TT - SKEW:
                    # final iteration: overlap the out-projection k=0..2
                    # partial accumulation (pairs 0..2 are normalized)
                    emit_p3a(i)
            if j == NP:
                for mt in range(TT - SKEW, TT):
                    emit_p3a(mt)
            if j >= 1:
                emit_evac(j - 1, cur_at)

        # ---- P3 tail: k=3 + bias, add the partial, store ------------------
        for mt in range(TT):
            ps = ps_a.tile([128, 512], f32, tag="slot", name="pp")
            nc.tensor.matmul(
                ps, mm(attnT_sb[KT - 1][:, mt * 128:(mt + 1) * 128]),
                mm(wo_sb[KT - 1]), start=True, stop=False)
            nc.tensor.matmul(ps, ones1, bo2_sb, start=False, stop=True)
            st = p_st.tile([128, E], f32, tag="st", name="st")
            nc.vector.tensor_add(st, ps, p3_part[mt])
            nc.sync.dma_start(out=out_d[mt * 128:(mt + 1) * 128, :], in_=st)

    nc.compile()
    return nc


def get_nc():
    if "nc" not in _cache:
        _cache["nc"] = _build_nc()
    return _cache["nc"]


def prep_inputs(query, pe, in_proj_weight, in_proj_bias, out_proj_weight,
                out_proj_bias):
    """Host-side sharding/layout prep. Returns per-core input maps."""
    query = np.asarray(query, dtype=np.float32)
    pe = np.asarray(pe, dtype=np.float32)
    in_proj_weight = np.asarray(in_proj_weight, dtype=np.float32)
    in_proj_bias = np.asarray(in_proj_bias, dtype=np.float32)
    out_proj_weight = np.asarray(out_proj_weight, dtype=np.float32)
    out_proj_bias = np.asarray(out_proj_bias, dtype=np.float32)

    def r32(x):
        # round-to-nearest-even onto e8m11 (fp32r keeps the top 20 bits)
        if MM_DT != "float32r":
            return np.ascontiguousarray(x, dtype=np.float32)
        u = np.ascontiguousarray(x, dtype=np.float32).view(np.uint32)
        u = (u + 0x7FF + ((u >> 12) & 1)) & np.uint32(0xFFFFF000)
        return u.view(np.float32)

    wqT = r32(in_proj_weight[0:E].T)                           # (E, E)
    wvT = r32(in_proj_weight[2 * E:3 * E].T)                   # (E, E)
    woT = r32(out_proj_weight.T)                               # (E, E)
    bq = np.ascontiguousarray(in_proj_bias[0:E])
    bv = in_proj_bias[2 * E:3 * E]
    bo2 = r32(out_proj_weight @ bv + out_proj_bias)

    in_maps = []
    for b in range(N_CORES):
        xT = r32(query[:, b, :].T)                             # (E, T)
        peT = np.ascontiguousarray(pe[b].T).astype(np.float16 if W_DT == "float16" else ml_dtypes.bfloat16)
        in_maps.append({
            "xT": xT, "peT": peT, "wqT": wqT, "wvT": wvT, "woT": woT,
            "bq": bq, "bo2": bo2, "ones1": np.ones(128, dtype=np.float32),
        })
    return in_maps


def kernel(query, pe, in_proj_weight, in_proj_bias, out_proj_weight,
           out_proj_bias):
    from concourse.bass_utils import run_bass_kernel_spmd

    nc = get_nc()
    in_maps = prep_inputs(query, pe, in_proj_weight, in_proj_bias,
                          out_proj_weight, out_proj_bias)
    res = run_bass_kernel_spmd(nc, in_maps, list(range(N_CORES)))
    out = np.empty((T, B, E), dtype=np.float32)
    for b in range(N_CORES):
        out[:, b, :] = res.results[b]["out"]
    return out


# revision 28
# speedup vs baseline: 1.1715x; 1.0196x over previous
"""DiffGraphTransformer attention kernel for 8x Trainium2 NeuronCores.

Reference computation (T=1024, B=8, E=512, H=8, hd=64):
    qkv = query @ in_proj_weight.T + in_proj_bias ; q,k,v = split(qkv)
    k = q ; q *= hd**-0.5
    per (batch,head): scores = q @ k.T            (T,T)
                      w = exp(scores - max) * pe[b]
                      w /= clip(sum(w,-1), 1e-6)
                      attn = w @ v
    out = attn @ out_proj_weight.T + out_proj_bias

Sharding: batch b -> core b.  Heads 8b..8b+7 all use pe[b], so each core is
fully independent (pure SPMD, no collectives, full inputs sharded on host).

Algebraic restructuring (exact up to fp rounding):
  * k == q, so the k-chunk of in_proj is dead weight; only Wq / Wv used.
  * softmax max-subtraction replaced by a constant shift (exp(s/8 - 10)):
    cancels in the normalization, keeps exp() inside fp16 range.
  * S = q q^T is symmetric (bit-identical across the diagonal since both
    matmul operands read the same qT buffer).  E = exp(S) stored [s, t]
    times pe^T gives W'[s,t] = w[t,s] - exactly the contraction-major
    operand the attention matmul needs; no (T,T) transpose ever happens.
  * attention lhsT = [v_h | ones] (128, 65): row 64 of the PSUM output is
    the softmax denominator, for free.
  * v-bias and out-bias fold into bo2 = Wo @ bv + bo (host precomputed),
    added via a K=1 ones matmul in the out-proj accumulation.
  * the pair's 4 denominator rows are DMA-reshaped to (32, 64) so ONE
    reciprocal covers them at 64 elems/lane (DVE recip is free-dim bound),
    then scattered through DRAM and partition-broadcast for the
    normalization multiply applied in SBUF.

Engine assignment: PE does projections/out-proj in float32r (e8m11, 1
cyc/row) and scores/attention in fp16; ACT does exp (the ~1us/(128,1024)
pacer); DVE does PSUM evacuations + most pe-multiplies (fp16 2x mode);
GPSIMD takes the last 2 pe-multiplies of each head; DMA handles the
reciprocal reshape/broadcast.  P2 is software-pipelined: iteration j
computes pair j's scores/exp/W' while the attention matmuls consume pair
j-1 (its W' tiles are all ready, so the PE never waits on ACT), with the
first SKEW attention k-steps hoisted above the scores to pad the iteration
boundary.  The out-projection is split: k=0..2 partials overlap the last
attention iteration; only k=3 + bias + store remain at the end.

Measured on trn2 (8 cores, whole kernel): 184.6 us, rel err 3.9e-4.
Known remaining headroom: the PE HAM clock gate spends roughly half the
kernel at 1.2 GHz - a single N=512 matmul stream shows only ~50% activity
at K=4/8 so it cannot re-warm itself; only concurrently-issued row-packed
score pairs push activity over the threshold.  Deeper scores PSUM rings
would fix the exp-lag stalls that trigger re-throttles, but PSUM (8 banks)
is exactly full.  Exploiting E's symmetry (exp only the triangle + DMA
transpose for the mirror) could cut the ACT exp load ~2x if ever needed.
"""

import sys

for _p in ("/opt/trn_rl_repo",):
    if _p not in sys.path:
        sys.path.insert(0, _p)

import numpy as np
import ml_dtypes

T, B, E = 1024, 8, 512
H = 8
HD = E // H  # 64
N_CORES = 8

# ---- tunables -------------------------------------------------------------
MM_DT = "float32r"  # dtype for PE matmuls: "float32r" | "float32" | "bfloat16"
W_DT = "float16"   # dtype of E / W' / v (the attention operands)
GPSIMD_MUL_TILES = 2  # of the 8 s-tiles of each head's W' multiply, how many go to GPSIMD
# ---------------------------------------------------------------------------

# global constant subtracted inside exp (cancels in normalization; keeps
# exp() outputs inside fp16 range: scores/8 - 10 is in [-16, ~6])
EXP_SHIFT = -10.0

_cache = {}


def _build_nc():
    import concourse.bass as bass
    import concourse.tile as tile
    import concourse.mybir as mybir
    from concourse import bacc
    from contextlib import ExitStack

    f32 = mybir.dt.float32
    bf16 = mybir.dt.bfloat16
    mm_dt = getattr(mybir.dt, MM_DT)
    w_dt = getattr(mybir.dt, W_DT)
    Exp = mybir.ActivationFunctionType.Exp

    nc = bacc.Bacc("TRN2", debug=False)

    # DRAM I/O (per-core contents supplied via in_maps)
    xT_d = nc.dram_tensor("xT", [E, T], mm_dt, kind="ExternalInput").ap()
    peT_d = nc.dram_tensor("peT", [T, T], w_dt, kind="ExternalInput").ap()
    wqT_d = nc.dram_tensor("wqT", [E, E], mm_dt, kind="ExternalInput").ap()
    wvT_d = nc.dram_tensor("wvT", [E, E], mm_dt, kind="ExternalInput").ap()
    woT_d = nc.dram_tensor("woT", [E, E], mm_dt, kind="ExternalInput").ap()
    bq_d = nc.dram_tensor("bq", [E], f32, kind="ExternalInput").ap()
    bo2_d = nc.dram_tensor("bo2", [E], mm_dt, kind="ExternalInput").ap()
    ones_d = nc.dram_tensor("ones1", [128], mm_dt, kind="ExternalInput").ap()
    out_d = nc.dram_tensor("out", [T, E], f32, kind="ExternalOutput").ap()

    KT = E // 128   # 4 contraction tiles for the projections
    TT = T // 128   # 8 t-tiles
    NH = T // 512   # 2 psum-bank halves of the t dimension

    def mm(ap):
        return ap

    with ExitStack() as ctx:
        tc = ctx.enter_context(tile.TileContext(nc))

        sing = ctx.enter_context(tc.tile_pool(name="sing", bufs=1))
        p_in = ctx.enter_context(tc.tile_pool(name="p_in", bufs=1))
        p_qv = ctx.enter_context(tc.tile_pool(name="p_qv", bufs=1))
        p_E = ctx.enter_context(tc.tile_pool(name="p_E", bufs=10))
        p_W = ctx.enter_context(tc.tile_pool(name="p_W", bufs=20))
        p_rc = ctx.enter_context(tc.tile_pool(name="p_rc", bufs=4))
        p_rm = ctx.enter_context(tc.tile_pool(name="p_rm", bufs=2))
        p_st = ctx.enter_context(tc.tile_pool(name="p_st", bufs=2))
        p_dr = ctx.enter_context(tc.tile_pool(name="p_dr", bufs=4, space="DRAM"))
        ps_a = ctx.enter_context(tc.tile_pool(name="ps_a", bufs=2, space="PSUM"))
        ps_b = ctx.enter_context(tc.tile_pool(name="ps_b", bufs=4, space="PSUM"))

        # ---- constants / weights into SBUF --------------------------------
        # DMA order matters: the first projection matmuls need xT + wq only,
        # so issue those first; wv arrives for the interleaved v-projection,
        # peT for the first pe-multiplies, and wo (out-proj) last.
        wq_sb = [sing.tile([128, E], mm_dt, tag=f"wq{k}", name="wq") for k in range(KT)]
        wv_sb = [sing.tile([128, E], mm_dt, tag=f"wv{k}", name="wv") for k in range(KT)]
        wo_sb = [sing.tile([128, E], mm_dt, tag=f"wo{k}", name="wo") for k in range(KT)]
        bq_sb = [sing.tile([128, 1], f32, tag=f"bq{k}", name="bq") for k in range(KT)]
        xT_sb = [p_in.tile([128, T], mm_dt, tag=f"xT{k}", name="xT") for k in range(KT)]
        peT_sb = [p_in.tile([128, T], w_dt, tag=f"peT{k}", name="peT") for k in range(TT)]
        for k in range(KT):
            nc.sync.dma_start(out=xT_sb[k], in_=xT_d[k * 128:(k + 1) * 128, :])
            nc.sync.dma_start(out=wq_sb[k], in_=wqT_d[k * 128:(k + 1) * 128, :])
            nc.sync.dma_start(out=bq_sb[k], in_=bq_d[k * 128:(k + 1) * 128].rearrange("(p one) -> p one", one=1))
        ebias = sing.tile([128, 1], f32, tag="ebias")
        nc.vector.memset(ebias, EXP_SHIFT)
        for k in range(KT):
            nc.sync.dma_start(out=wv_sb[k], in_=wvT_d[k * 128:(k + 1) * 128, :])
        for i in range(TT):
            nc.sync.dma_start(out=peT_sb[i], in_=peT_d[i * 128:(i + 1) * 128, :])
        ones1 = sing.tile([1, 128], mm_dt, tag="ones1")
        nc.sync.dma_start(out=ones1, in_=ones_d.unsqueeze(0))
        bo2_sb = sing.tile([1, E], mm_dt, tag="bo2")
        nc.sync.dma_start(out=bo2_sb, in_=bo2_d.unsqueeze(0))
        for k in range(KT):
            nc.sync.dma_start(out=wo_sb[k], in_=woT_d[k * 128:(k + 1) * 128, :])

        # ---- P1: projections ----------------------------------------------
        # qT[e_out, t] with e_out on partitions (4 tiles); includes q-bias.
        qT_sb = [p_qv.tile([128, T], w_dt, tag=f"qT{k}", name="qT") for k in range(KT)]
        # v[t, e_out] natural, with a ones column appended per head:
        # layout (128, 8*65): head h occupies cols [65h, 65h+64), ones at 65h+64.
        v_sb = [p_qv.tile([128, H * (HD + 1)], w_dt, tag=f"v{k}", name="v") for k in range(TT)]

        def emit_qproj(m, nh):
            ps = ps_b.tile([128, 512], f32, tag="slot", name="pp")
            for k in range(KT):
                nc.tensor.matmul(
                    ps, mm(wq_sb[k][:, m * 128:(m + 1) * 128]),
                    mm(xT_sb[k][:, nh * 512:(nh + 1) * 512]),
                    start=(k == 0), stop=(k == KT - 1))
            nc.vector.tensor_scalar_add(
                qT_sb[m][:, nh * 512:(nh + 1) * 512], ps, bq_sb[m])

        def emit_vproj(mt):
            ps = ps_b.tile([128, 512], f32, tag="slot", name="pp")
            for k in range(KT):
                nc.tensor.matmul(
                    ps, mm(xT_sb[k][:, mt * 128:(mt + 1) * 128]), mm(wv_sb[k]),
                    start=(k == 0), stop=(k == KT - 1))
            v_dst = v_sb[mt].rearrange("p (h c) -> p h c", c=HD + 1)
            nc.vector.tensor_copy(
                v_dst[:, :, 0:HD],
                ps.rearrange("p (h c) -> p h c", c=HD))
            nc.vector.memset(v_dst[:, :, HD:HD + 1], 1.0)

        # pair 0's qT upfront; the other 14 projection groups interleave
        # with iteration 0's scores so ACT starts exp'ing ~20us earlier and
        # the packable fp16 score pairs warm the PE clock gate early.
        for nh in range(NH):
            emit_qproj(0, nh)
        proj_rest = [("q", m, nh) for m in range(1, KT) for nh in range(NH)]
        proj_rest += [("v", mt, None) for mt in range(TT)]

        # ---- P2: attention, software-pipelined over head pairs ------------
        # Iteration j produces W' for pair j (scores -> exp -> pe-multiply)
        # while the attention matmuls consume pair j-1's W' tiles.  This
        # keeps the PE stream dense across pair boundaries (HAM stays warm).
        attnT_sb = [p_qv.tile([128, T], mm_dt, tag=f"attnT{k}", name="attnT") for k in range(KT)]
        NP = H // 2  # pairs
        Ws_of = {}   # pair j -> [hh][i] W' tiles

        def emit_scores(j, i):
            # nh-major emission: consecutive MMs alternate row groups
            # (head A rows 0-63, head B rows 64-127) so the PE runs the
            # pair concurrently and overlaps B's LDWEIGHTS with A's MM.
            qt = qT_sb[j]
            scs = [ps_a.tile([128, T], f32, tag="slot", name="sc") for _ in range(2)]
            for nh in range(NH):
                for hh in range(2):
                    r0 = hh * HD
                    nc.tensor.matmul(
                        scs[hh][:, nh * 512:(nh + 1) * 512],
                        mm(qt[r0:r0 + HD, i * 128:(i + 1) * 128]),
                        mm(qt[r0:r0 + HD, nh * 512:(nh + 1) * 512]),
                        start=True, stop=True,
                        tile_position=(r0, 0))
            for hh in range(2):
                Et = p_E.tile([128, T], w_dt, tag="E", name="Et")
                nc.scalar.activation(out=Et, in_=scs[hh], func=Exp, scale=0.125, bias=ebias)
                Wt = p_W.tile([128, T], w_dt, tag="W", name="Wt")
                if i >= TT - GPSIMD_MUL_TILES:
                    nc.gpsimd.tensor_mul(Wt, Et, peT_sb[i])
                else:
                    nc.vector.tensor_mul(Wt, Et, peT_sb[i])
                Ws_of[j][hh][i] = Wt

        def emit_attn_kstep(j, i, at_ps):
            for hh in range(2):
                vcol = 65 * (2 * j + hh)
                for nh in range(NH):
                    nc.tensor.matmul(
                        at_ps[hh][nh],
                        v_sb[i][:, vcol:vcol + HD + 1],
                        Ws_of[j][hh][i][:, nh * 512:(nh + 1) * 512],
                        start=(i == 0), stop=(i == TT - 1))

        def emit_evac(j, at_ps):
            # plain evacuation first (frees the attention PSUM slots fast);
            # normalization happens afterwards in SBUF, in place.
            for hh in range(2):
                for nh in range(NH):
                    nc.vector.tensor_copy(
                        attnT_sb[j][hh * HD:(hh + 1) * HD,
                                    nh * 512:(nh + 1) * 512],
                        at_ps[hh][nh][0:HD, :])
            rows = p_rc.tile([HD + 1, 4 * 512], f32, tag="rc", name="rows", bufs=2)
            for hh in range(2):
                for nh in range(NH):
                    r = hh * NH + nh
                    nc.vector.tensor_copy(
                        rows[HD:HD + 1, r * 512:(r + 1) * 512],
                        at_ps[hh][nh][HD:HD + 1, :])
            # one low-free-dim reciprocal for the pair's 4 denominator rows
            rg = p_rc.tile([32, 64], f32, tag="rg", name="rg")
            nc.sync.dma_start(
                out=rg,
                in_=rows[HD:HD + 1, :].rearrange("one (a c) -> one a c", c=64))
            rgi = p_rc.tile([32, 64], f32, tag="rgi", name="rgi")
            nc.vector.reciprocal(rgi, rg)
            rrow = p_dr.tile([1, 4 * 512], f32, tag="rrow", name="rrow")
            nc.sync.dma_start(
                out=rrow.rearrange("one (a c) -> one a c", c=64), in_=rgi)
            for hh in range(2):
                for nh in range(NH):
                    r = hh * NH + nh
                    rm = p_rm.tile([128, 512], f32, tag="rm", name="rm")
                    rms = rm[hh * HD:(hh + 1) * HD, :]
                    seg = rrow[0:1, r * 512:(r + 1) * 512]
                    bcast = bass.AP(tensor=seg.tensor, offset=seg.offset,
                                    ap=[[0, HD]] + list(seg.ap[1:]))
                    nc.sync.dma_start(out=rms, in_=bcast)
                    dst = attnT_sb[j][hh * HD:(hh + 1) * HD,
                                      nh * 512:(nh + 1) * 512]
                    nc.vector.tensor_mul(dst, dst, rms)

        p3_part = [p_qv.tile([128, E], f32, tag=f"p3p{k}", name="p3p")
                   for k in range(TT)]

        def emit_p3a(mt):
            # out-projection partial over k=0..2 (heads 0..5); evacuated to
            # SBUF so the PSUM slot recycles while pair 3 finishes.
            ps = ps_a.tile([128, 512], f32, tag="slot", name="pp")
            for k in range(KT - 1):
                nc.tensor.matmul(
                    ps, mm(attnT_sb[k][:, mt * 128:(mt + 1) * 128]),
                    mm(wo_sb[k]), start=(k == 0), stop=(k == KT - 2))
            nc.vector.tensor_copy(p3_part[mt], ps)

        # Skewed schedule: at each iteration boundary the first 3 attention
        # k-steps (which need no ACT results) run before the first scores
        # matmul, absorbing the exp-lag so the PE stream never gaps long
        # enough for the HAM clock gate to re-throttle.
        SKEW = 3
        for j in range(NP + 1):
            if j < NP:
                Ws_of[j] = [[None] * TT, [None] * TT]
                if j >= 2:
                    del Ws_of[j - 2]
            cur_at = None
            if j >= 1:
                cur_at = [[ps_b.tile([HD + 1, 512], f32, tag="slot", name="at")
                           for _ in range(NH)] for _ in range(2)]
                for i in range(SKEW):
                    emit_attn_kstep(j - 1, i, cur_at)
            for i in range(TT):
                if j < NP:
                    emit_scores(j, i)
                if j == 0:
                    for _ in range(2):
                        if proj_rest:
                            kind, a1, a2 = proj_rest.pop(0)
                            if kind == "q":
                                emit_qproj(a1, a2)
                            else:
                                emit_vproj(a1)
                if j >= 1 and i + SKEW < TT:
                    emit_attn_kstep(j - 1, i + SKEW, cur_at)
                if j == NP and i < TT - SKEW:
                    # final iteration: overlap the out-projection k=0..2
                    # partial accumulation (pairs 0..2 are normalized)
                    emit_p3a(i)
            if j == NP:
                for mt in range(TT - SKEW, TT):
                    emit_p3a(mt)
            if j >= 1:
                emit_evac(j - 1, cur_at)

        # ---- P3 tail: k=3 + bias, add the partial, store ------------------
        for mt in range(TT):
            ps = ps_a.tile([128, 512], f32, tag="slot", name="pp")
            nc.tensor.matmul(
                ps, mm(attnT_sb[KT - 1][:, mt * 128:(mt + 1) * 128]),
                mm(wo_sb[KT - 1]), start=True, stop=False)
            nc.tensor.matmul(ps, ones1, bo2_sb, start=False, stop=True)
            st = p_st.tile([128, E], f32, tag="st", name="st")
            nc.vector.tensor_add(st, ps, p3_part[mt])
            nc.sync.dma_start(out=out_d[mt * 128:(mt + 1) * 128, :], in_=st)

    nc.compile()
    return nc


def get_nc():
    if "nc" not in _cache:
        _cache["nc"] = _build_nc()
    return _cache["nc"]


def prep_inputs(query, pe, in_proj_weight, in_proj_bias, out_proj_weight,
                out_proj_bias):
    """Host-side sharding/layout prep. Returns per-core input maps."""
    query = np.asarray(query, dtype=np.float32)
    pe = np.asarray(pe, dtype=np.float32)
    in_proj_weight = np.asarray(in_proj_weight, dtype=np.float32)
    in_proj_bias = np.asarray(in_proj_bias, dtype=np.float32)
    out_proj_weight = np.asarray(out_proj_weight, dtype=np.float32)
    out_proj_bias = np.asarray(out_proj_bias, dtype=np.float32)

    def r32(x):
        # round-to-nearest-even onto e8m11 (fp32r keeps the top 20 bits)
        if MM_DT != "float32r":
            return np.ascontiguousarray(x, dtype=np.float32)
        u = np.ascontiguousarray(x, dtype=np.float32).view(np.uint32)
        u = (u + 0x7FF + ((u >> 12) & 1)) & np.uint32(0xFFFFF000)
        return u.view(np.float32)

    wqT = r32(in_proj_weight[0:E].T)                           # (E, E)
    wvT = r32(in_proj_weight[2 * E:3 * E].T)                   # (E, E)
    woT = r32(out_proj_weight.T)                               # (E, E)
    bq = np.ascontiguousarray(in_proj_bias[0:E])
    bv = in_proj_bias[2 * E:3 * E]
    bo2 = r32(out_proj_weight @ bv + out_proj_bias)

    in_maps = []
    for b in range(N_CORES):
        xT = r32(query[:, b, :].T)                             # (E, T)
        peT = np.ascontiguousarray(pe[b].T).astype(np.float16 if W_DT == "float16" else ml_dtypes.bfloat16)
        in_maps.append({
            "xT": xT, "peT": peT, "wqT": wqT, "wvT": wvT, "woT": woT,
            "bq": bq, "bo2": bo2, "ones1": np.ones(128, dtype=np.float32),
        })
    return in_maps


def kernel(query, pe, in_proj_weight, in_proj_bias, out_proj_weight,
           out_proj_bias):
    from concourse.bass_utils import run_bass_kernel_spmd

    nc = get_nc()
    in_maps = prep_inputs(query, pe, in_proj_weight, in_proj_bias,
                          out_proj_weight, out_proj_bias)
    res = run_bass_kernel_spmd(nc, in_maps, list(range(N_CORES)))
    out = np.empty((T, B, E), dtype=np.float32)
    for b in range(N_CORES):
        out[:, b, :] = res.results[b]["out"]
    return out


# revision 30
# speedup vs baseline: 1.1940x; 1.0192x over previous
"""DiffGraphTransformer attention kernel for 8x Trainium2 NeuronCores.

Reference computation (T=1024, B=8, E=512, H=8, hd=64):
    qkv = query @ in_proj_weight.T + in_proj_bias ; q,k,v = split(qkv)
    k = q ; q *= hd**-0.5
    per (batch,head): scores = q @ k.T            (T,T)
                      w = exp(scores - max) * pe[b]
                      w /= clip(sum(w,-1), 1e-6)
                      attn = w @ v
    out = attn @ out_proj_weight.T + out_proj_bias

Sharding: batch b -> core b.  Heads 8b..8b+7 all use pe[b], so each core is
fully independent (pure SPMD, no collectives, full inputs sharded on host).

Algebraic restructuring (exact up to fp rounding):
  * k == q, so the k-chunk of in_proj is dead weight; only Wq / Wv used.
  * softmax max-subtraction replaced by a constant shift (exp(s/8 - 10)):
    cancels in the normalization, keeps exp() inside fp16 range.
  * S = q q^T is symmetric (bit-identical across the diagonal since both
    matmul operands read the same qT buffer).  E = exp(S) stored [s, t]
    times pe^T gives W'[s,t] = w[t,s] - exactly the contraction-major
    operand the attention matmul needs; no (T,T) transpose ever happens.
  * attention lhsT = [v_h | ones] (128, 65): row 64 of the PSUM output is
    the softmax denominator, for free.
  * v-bias and out-bias fold into bo2 = Wo @ bv + bo (host precomputed),
    added via a K=1 ones matmul in the out-proj accumulation.
  * the pair's 4 denominator rows are DMA-reshaped to (32, 64) so ONE
    reciprocal covers them at 64 elems/lane (DVE recip is free-dim bound),
    then scattered through DRAM and partition-broadcast for the
    normalization multiply applied in SBUF.

Engine assignment: PE does projections/out-proj in float32r (e8m11, 1
cyc/row) and scores/attention in fp16; ACT does exp (the ~1us/(128,1024)
pacer); DVE does PSUM evacuations + most pe-multiplies (fp16 2x mode);
GPSIMD takes the last 2 pe-multiplies of each head; DMA handles the
reciprocal reshape/broadcast.  P2 is software-pipelined: iteration j
computes pair j's scores/exp/W' while the attention matmuls consume pair
j-1 (its W' tiles are all ready, so the PE never waits on ACT), with the
first SKEW attention k-steps hoisted above the scores to pad the iteration
boundary.  The out-projection is split: k=0..2 partials overlap the last
attention iteration; only k=3 + bias + store remain at the end.

Measured on trn2 (8 cores, whole kernel): 164.4 us, rel err 3.9e-4.
Known remaining headroom: the PE HAM clock gate spends roughly half the
kernel at 1.2 GHz - a single N=512 matmul stream shows only ~50% activity
at K=4/8 so it cannot re-warm itself; only concurrently-issued row-packed
score pairs push activity over the threshold.  Deeper scores PSUM rings
would fix the exp-lag stalls that trigger re-throttles, but PSUM (8 banks)
is exactly full.  Exploiting E's symmetry (exp only the triangle + DMA
transpose for the mirror) could cut the ACT exp load ~2x if ever needed.
"""

import sys

for _p in ("/opt/trn_rl_repo",):
    if _p not in sys.path:
        sys.path.insert(0, _p)

import numpy as np
import ml_dtypes

T, B, E = 1024, 8, 512
H = 8
HD = E // H  # 64
N_CORES = 8

# ---- tunables -------------------------------------------------------------
MM_DT = "float32r"  # dtype for PE matmuls: "float32r" | "float32" | "bfloat16"
W_DT = "float16"   # dtype of E / W' / v (the attention operands)
GPSIMD_MUL_TILES = 2  # of the 8 s-tiles of each head's W' multiply, how many go to GPSIMD
# ---------------------------------------------------------------------------

# global constant subtracted inside exp (cancels in normalization; keeps
# exp() outputs inside fp16 range: scores/8 - 10 is in [-16, ~6])
EXP_SHIFT = -10.0

_cache = {}


def _build_nc():
    import concourse.bass as bass
    import concourse.tile as tile
    import concourse.mybir as mybir
    from concourse import bacc
    from contextlib import ExitStack

    f32 = mybir.dt.float32
    bf16 = mybir.dt.bfloat16
    mm_dt = getattr(mybir.dt, MM_DT)
    w_dt = getattr(mybir.dt, W_DT)
    Exp = mybir.ActivationFunctionType.Exp

    nc = bacc.Bacc("TRN2", debug=False)

    # DRAM I/O (per-core contents supplied via in_maps)
    xT_d = nc.dram_tensor("xT", [E, T], mm_dt, kind="ExternalInput").ap()
    peT_d = nc.dram_tensor("peT", [T, T], w_dt, kind="ExternalInput").ap()
    wqT_d = nc.dram_tensor("wqT", [E, E], mm_dt, kind="ExternalInput").ap()
    wvT_d = nc.dram_tensor("wvT", [E, E], mm_dt, kind="ExternalInput").ap()
    woT_d = nc.dram_tensor("woT", [E, E], mm_dt, kind="ExternalInput").ap()
    bq_d = nc.dram_tensor("bq", [E], f32, kind="ExternalInput").ap()
    bo2_d = nc.dram_tensor("bo2", [E], mm_dt, kind="ExternalInput").ap()
    ones_d = nc.dram_tensor("ones1", [128], mm_dt, kind="ExternalInput").ap()
    out_d = nc.dram_tensor("out", [T, E], f32, kind="ExternalOutput").ap()

    KT = E // 128   # 4 contraction tiles for the projections
    TT = T // 128   # 8 t-tiles
    NH = T // 512   # 2 psum-bank halves of the t dimension

    def mm(ap):
        return ap

    with ExitStack() as ctx:
        tc = ctx.enter_context(tile.TileContext(nc))

        sing = ctx.enter_context(tc.tile_pool(name="sing", bufs=1))
        p_in = ctx.enter_context(tc.tile_pool(name="p_in", bufs=1))
        p_qv = ctx.enter_context(tc.tile_pool(name="p_qv", bufs=1))
        p_E = ctx.enter_context(tc.tile_pool(name="p_E", bufs=10))
        p_W = ctx.enter_context(tc.tile_pool(name="p_W", bufs=20))
        p_rc = ctx.enter_context(tc.tile_pool(name="p_rc", bufs=4))
        p_rm = ctx.enter_context(tc.tile_pool(name="p_rm", bufs=2))
        p_st = ctx.enter_context(tc.tile_pool(name="p_st", bufs=2))
        p_dr = ctx.enter_context(tc.tile_pool(name="p_dr", bufs=4, space="DRAM"))
        ps_a = ctx.enter_context(tc.tile_pool(name="ps_a", bufs=2, space="PSUM"))
        ps_b = ctx.enter_context(tc.tile_pool(name="ps_b", bufs=4, space="PSUM"))

        # ---- constants / weights into SBUF --------------------------------
        # DMA order matters: the first projection matmuls need xT + wq only,
        # so issue those first; wv arrives for the interleaved v-projection,
        # peT for the first pe-multiplies, and wo (out-proj) last.
        wq_sb = [sing.tile([128, E], mm_dt, tag=f"wq{k}", name="wq") for k in range(KT)]
        wv_sb = [sing.tile([128, E], mm_dt, tag=f"wv{k}", name="wv") for k in range(KT)]
        wo_sb = [sing.tile([128, E], mm_dt, tag=f"wo{k}", name="wo") for k in range(KT)]
        bq_sb = [sing.tile([128, 1], f32, tag=f"bq{k}", name="bq") for k in range(KT)]
        xT_sb = [p_in.tile([128, T], mm_dt, tag=f"xT{k}", name="xT") for k in range(KT)]
        peT_sb = [p_in.tile([128, T], w_dt, tag=f"peT{k}", name="peT") for k in range(TT)]
        for k in range(KT):
            nc.sync.dma_start(out=xT_sb[k], in_=xT_d[k * 128:(k + 1) * 128, :])
            nc.sync.dma_start(out=wq_sb[k], in_=wqT_d[k * 128:(k + 1) * 128, :])
            nc.sync.dma_start(out=bq_sb[k], in_=bq_d[k * 128:(k + 1) * 128].rearrange("(p one) -> p one", one=1))
        ebias = sing.tile([128, 1], f32, tag="ebias")
        nc.vector.memset(ebias, EXP_SHIFT)
        for k in range(KT):
            nc.sync.dma_start(out=wv_sb[k], in_=wvT_d[k * 128:(k + 1) * 128, :])
        for i in range(TT):
            nc.sync.dma_start(out=peT_sb[i], in_=peT_d[i * 128:(i + 1) * 128, :])
        ones1 = sing.tile([1, 128], mm_dt, tag="ones1")
        nc.sync.dma_start(out=ones1, in_=ones_d.unsqueeze(0))
        bo2_sb = sing.tile([1, E], mm_dt, tag="bo2")
        nc.sync.dma_start(out=bo2_sb, in_=bo2_d.unsqueeze(0))
        for k in range(KT):
            nc.sync.dma_start(out=wo_sb[k], in_=woT_d[k * 128:(k + 1) * 128, :])

        # ---- P1: projections ----------------------------------------------
        # qT[e_out, t] with e_out on partitions (4 tiles); includes q-bias.
        qT_sb = [p_qv.tile([128, T], w_dt, tag=f"qT{k}", name="qT") for k in range(KT)]
        # v[t, e_out] natural, with a ones column appended per head:
        # layout (128, 8*65): head h occupies cols [65h, 65h+64), ones at 65h+64.
        v_sb = [p_qv.tile([128, H * (HD + 1)], w_dt, tag=f"v{k}", name="v") for k in range(TT)]

        def emit_qproj(m, nh):
            ps = ps_b.tile([128, 512], f32, tag="slot", name="pp")
            for k in range(KT):
                nc.tensor.matmul(
                    ps, mm(wq_sb[k][:, m * 128:(m + 1) * 128]),
                    mm(xT_sb[k][:, nh * 512:(nh + 1) * 512]),
                    start=(k == 0), stop=(k == KT - 1))
            nc.vector.tensor_scalar_add(
                qT_sb[m][:, nh * 512:(nh + 1) * 512], ps, bq_sb[m])

        def emit_vproj(mt):
            ps = ps_b.tile([128, 512], f32, tag="slot", name="pp")
            for k in range(KT):
                nc.tensor.matmul(
                    ps, mm(xT_sb[k][:, mt * 128:(mt + 1) * 128]), mm(wv_sb[k]),
                    start=(k == 0), stop=(k == KT - 1))
            v_dst = v_sb[mt].rearrange("p (h c) -> p h c", c=HD + 1)
            nc.vector.tensor_copy(
                v_dst[:, :, 0:HD],
                ps.rearrange("p (h c) -> p h c", c=HD))
            nc.vector.memset(v_dst[:, :, HD:HD + 1], 1.0)

        # pair 0's qT upfront; the other 14 projection groups interleave
        # with iteration 0's scores so ACT starts exp'ing ~20us earlier and
        # the packable fp16 score pairs warm the PE clock gate early.
        for nh in range(NH):
            emit_qproj(0, nh)
        proj_rest = [("q", m, nh) for m in range(1, KT) for nh in range(NH)]
        proj_rest += [("v", mt, None) for mt in range(TT)]

        # ---- P2: attention, software-pipelined over head pairs ------------
        # Iteration j produces W' for pair j (scores -> exp -> pe-multiply)
        # while the attention matmuls consume pair j-1's W' tiles.  This
        # keeps the PE stream dense across pair boundaries (HAM stays warm).
        attnT_sb = [p_qv.tile([128, T], mm_dt, tag=f"attnT{k}", name="attnT") for k in range(KT)]
        NP = H // 2  # pairs
        Ws_of = {}   # pair j -> [hh][i] W' tiles

        def emit_scores(j, i):
            # nh-major emission: consecutive MMs alternate row groups
            # (head A rows 0-63, head B rows 64-127) so the PE runs the
            # pair concurrently and overlaps B's LDWEIGHTS with A's MM.
            qt = qT_sb[j]
            scs = [ps_a.tile([128, T], f32, tag="slot", name="sc") for _ in range(2)]
            for nh in range(NH):
                for hh in range(2):
                    r0 = hh * HD
                    nc.tensor.matmul(
                        scs[hh][:, nh * 512:(nh + 1) * 512],
                        mm(qt[r0:r0 + HD, i * 128:(i + 1) * 128]),
                        mm(qt[r0:r0 + HD, nh * 512:(nh + 1) * 512]),
                        start=True, stop=True,
                        tile_position=(r0, 0))
            for hh in range(2):
                Et = p_E.tile([128, T], w_dt, tag="E", name="Et")
                nc.scalar.activation(out=Et, in_=scs[hh], func=Exp, scale=0.125, bias=ebias)
                Wt = p_W.tile([128, T], w_dt, tag="W", name="Wt")
                if i >= TT - GPSIMD_MUL_TILES:
                    nc.gpsimd.tensor_mul(Wt, Et, peT_sb[i])
                else:
                    nc.vector.tensor_mul(Wt, Et, peT_sb[i])
                Ws_of[j][hh][i] = Wt

        def emit_attn_kstep(j, i, at_ps):
            for hh in range(2):
                vcol = 65 * (2 * j + hh)
                for nh in range(NH):
                    nc.tensor.matmul(
                        at_ps[hh][nh],
                        v_sb[i][:, vcol:vcol + HD + 1],
                        Ws_of[j][hh][i][:, nh * 512:(nh + 1) * 512],
                        start=(i == 0), stop=(i == TT - 1))

        def emit_evac(j, at_ps):
            # plain evacuation first (frees the attention PSUM slots fast);
            # normalization happens afterwards in SBUF, in place.
            for hh in range(2):
                for nh in range(NH):
                    nc.vector.tensor_copy(
                        attnT_sb[j][hh * HD:(hh + 1) * HD,
                                    nh * 512:(nh + 1) * 512],
                        at_ps[hh][nh][0:HD, :])
            rows = p_rc.tile([HD + 1, 4 * 512], f32, tag="rc", name="rows", bufs=2)
            for hh in range(2):
                for nh in range(NH):
                    r = hh * NH + nh
                    nc.vector.tensor_copy(
                        rows[HD:HD + 1, r * 512:(r + 1) * 512],
                        at_ps[hh][nh][HD:HD + 1, :])
            # one low-free-dim reciprocal for the pair's 4 denominator rows
            rg = p_rc.tile([32, 64], f32, tag="rg", name="rg")
            nc.sync.dma_start(
                out=rg,
                in_=rows[HD:HD + 1, :].rearrange("one (a c) -> one a c", c=64))
            rgi = p_rc.tile([32, 64], f32, tag="rgi", name="rgi")
            nc.vector.reciprocal(rgi, rg)
            rrow = p_dr.tile([1, 4 * 512], f32, tag="rrow", name="rrow")
            nc.sync.dma_start(
                out=rrow.rearrange("one (a c) -> one a c", c=64), in_=rgi)
            for hh in range(2):
                for nh in range(NH):
                    r = hh * NH + nh
                    rm = p_rm.tile([128, 512], f32, tag="rm", name="rm")
                    rms = rm[hh * HD:(hh + 1) * HD, :]
                    seg = rrow[0:1, r * 512:(r + 1) * 512]
                    bcast = bass.AP(tensor=seg.tensor, offset=seg.offset,
                                    ap=[[0, HD]] + list(seg.ap[1:]))
                    nc.sync.dma_start(out=rms, in_=bcast)
                    dst = attnT_sb[j][hh * HD:(hh + 1) * HD,
                                      nh * 512:(nh + 1) * 512]
                    nc.vector.tensor_mul(dst, dst, rms)

        p3_part = [p_qv.tile([128, E], f32, tag=f"p3p{k}", name="p3p")
                   for k in range(TT)]

        def emit_p3a(mt):
            # out-projection partial over k=0..2 (heads 0..5); evacuated to
            # SBUF so the PSUM slot recycles while pair 3 finishes.
            ps = ps_a.tile([128, 512], f32, tag="slot", name="pp")
            for k in range(KT - 1):
                nc.tensor.matmul(
                    ps, mm(attnT_sb[k][:, mt * 128:(mt + 1) * 128]),
                    mm(wo_sb[k]), start=(k == 0), stop=(k == KT - 2))
            nc.vector.tensor_copy(p3_part[mt], ps)

        # Skewed schedule: at each iteration boundary the first 3 attention
        # k-steps (which need no ACT results) run before the first scores
        # matmul, absorbing the exp-lag so the PE stream never gaps long
        # enough for the HAM clock gate to re-throttle.
        SKEW = 3
        for j in range(NP + 1):
            if j < NP:
                Ws_of[j] = [[None] * TT, [None] * TT]
                if j >= 2:
                    del Ws_of[j - 2]
            cur_at = None
            if j >= 1:
                cur_at = [[ps_b.tile([HD + 1, 512], f32, tag="slot", name="at")
                           for _ in range(NH)] for _ in range(2)]
                for i in range(SKEW):
                    emit_attn_kstep(j - 1, i, cur_at)
            if j == NP:
                # final iteration: finish the attention immediately and start
                # the pair-3 normalization chain; the out-projection k=0..2
                # partials keep the PE busy while the chain completes.
                for i in range(SKEW, TT):
                    emit_attn_kstep(j - 1, i, cur_at)
                emit_evac(j - 1, cur_at)
                for mt in range(TT):
                    emit_p3a(mt)
            else:
                for i in range(TT):
                    emit_scores(j, i)
                    if j == 0:
                        for _ in range(2):
                            if proj_rest:
                                kind, a1, a2 = proj_rest.pop(0)
                                if kind == "q":
                                    emit_qproj(a1, a2)
                                else:
                                    emit_vproj(a1)
                    if j >= 1 and i + SKEW < TT:
                        emit_attn_kstep(j - 1, i + SKEW, cur_at)
                if j >= 1:
                    emit_evac(j - 1, cur_at)

        # ---- P3 tail: k=3 + bias, add the partial, store ------------------
        for mt in range(TT):
            ps = ps_a.tile([128, 512], f32, tag="slot", name="pp")
            nc.tensor.matmul(
                ps, mm(attnT_sb[KT - 1][:, mt * 128:(mt + 1) * 128]),
                mm(wo_sb[KT - 1]), start=True, stop=False)
            nc.tensor.matmul(ps, ones1, bo2_sb, start=False, stop=True)
            st = p_st.tile([128, E], f32, tag="st", name="st")
            nc.vector.tensor_add(st, ps, p3_part[mt])
            nc.sync.dma_start(out=out_d[mt * 128:(mt + 1) * 128, :], in_=st)

    nc.compile()
    return nc


def get_nc():
    if "nc" not in _cache:
        _cache["nc"] = _build_nc()
    return _cache["nc"]


def prep_inputs(query, pe, in_proj_weight, in_proj_bias, out_proj_weight,
                out_proj_bias):
    """Host-side sharding/layout prep. Returns per-core input maps."""
    query = np.asarray(query, dtype=np.float32)
    pe = np.asarray(pe, dtype=np.float32)
    in_proj_weight = np.asarray(in_proj_weight, dtype=np.float32)
    in_proj_bias = np.asarray(in_proj_bias, dtype=np.float32)
    out_proj_weight = np.asarray(out_proj_weight, dtype=np.float32)
    out_proj_bias = np.asarray(out_proj_bias, dtype=np.float32)

    def r32(x):
        # round-to-nearest-even onto e8m11 (fp32r keeps the top 20 bits)
        if MM_DT != "float32r":
            return np.ascontiguousarray(x, dtype=np.float32)
        u = np.ascontiguousarray(x, dtype=np.float32).view(np.uint32)
        u = (u + 0x7FF + ((u >> 12) & 1)) & np.uint32(0xFFFFF000)
        return u.view(np.float32)

    wqT = r32(in_proj_weight[0:E].T)                           # (E, E)
    wvT = r32(in_proj_weight[2 * E:3 * E].T)                   # (E, E)
    woT = r32(out_proj_weight.T)                               # (E, E)
    bq = np.ascontiguousarray(in_proj_bias[0:E])
    bv = in_proj_bias[2 * E:3 * E]
    bo2 = r32(out_proj_weight @ bv + out_proj_bias)

    in_maps = []
    for b in range(N_CORES):
        xT = r32(query[:, b, :].T)                             # (E, T)
        peT = np.ascontiguousarray(pe[b].T).astype(np.float16 if W_DT == "float16" else ml_dtypes.bfloat16)
        in_maps.append({
            "xT": xT, "peT": peT, "wqT": wqT, "wvT": wvT, "woT": woT,
            "bq": bq, "bo2": bo2, "ones1": np.ones(128, dtype=np.float32),
        })
    return in_maps


def kernel(query, pe, in_proj_weight, in_proj_bias, out_proj_weight,
           out_proj_bias):
    from concourse.bass_utils import run_bass_kernel_spmd

    nc = get_nc()
    in_maps = prep_inputs(query, pe, in_proj_weight, in_proj_bias,
                          out_proj_weight, out_proj_bias)
    res = run_bass_kernel_spmd(nc, in_maps, list(range(N_CORES)))
    out = np.empty((T, B, E), dtype=np.float32)
    for b in range(N_CORES):
        out[:, b, :] = res.results[b]["out"]
    return out


# revision 32
# speedup vs baseline: 1.2159x; 1.0184x over previous
"""DiffGraphTransformer attention kernel for 8x Trainium2 NeuronCores.

Reference computation (T=1024, B=8, E=512, H=8, hd=64):
    qkv = query @ in_proj_weight.T + in_proj_bias ; q,k,v = split(qkv)
    k = q ; q *= hd**-0.5
    per (batch,head): scores = q @ k.T            (T,T)
                      w = exp(scores - max) * pe[b]
                      w /= clip(sum(w,-1), 1e-6)
                      attn = w @ v
    out = attn @ out_proj_weight.T + out_proj_bias

Sharding: batch b -> core b.  Heads 8b..8b+7 all use pe[b], so each core is
fully independent (pure SPMD, no collectives, full inputs sharded on host).

Algebraic restructuring (exact up to fp rounding):
  * k == q, so the k-chunk of in_proj is dead weight; only Wq / Wv used.
  * softmax max-subtraction replaced by a constant shift (exp(s/8 - 10)):
    cancels in the normalization, keeps exp() inside fp16 range.
  * S = q q^T is symmetric (bit-identical across the diagonal since both
    matmul operands read the same qT buffer).  E = exp(S) stored [s, t]
    times pe^T gives W'[s,t] = w[t,s] - exactly the contraction-major
    operand the attention matmul needs; no (T,T) transpose ever happens.
  * attention lhsT = [v_h | ones] (128, 65): row 64 of the PSUM output is
    the softmax denominator, for free.
  * v-bias and out-bias fold into bo2 = Wo @ bv + bo (host precomputed),
    added via a K=1 ones matmul in the out-proj accumulation.
  * the pair's 4 denominator rows are DMA-reshaped to (32, 64) so ONE
    reciprocal covers them at 64 elems/lane (DVE recip is free-dim bound),
    then scattered through DRAM and partition-broadcast for the
    normalization multiply applied in SBUF.

Engine assignment: PE does projections/out-proj in float32r (e8m11, 1
cyc/row) and scores/attention in fp16; ACT does exp (the ~1us/(128,1024)
pacer); DVE does PSUM evacuations + most pe-multiplies (fp16 2x mode);
GPSIMD takes the last 2 pe-multiplies of each head; DMA handles the
reciprocal reshape/broadcast.  P2 is software-pipelined: iteration j
computes pair j's scores/exp/W' while the attention matmuls consume pair
j-1 (its W' tiles are all ready, so the PE never waits on ACT), with the
first SKEW attention k-steps hoisted above the scores to pad the iteration
boundary.  The final iteration runs its attention k-steps densely and starts the
normalization chain immediately, with the out-projection k=0..2
partials as PE filler; only k=3 + bias + store remain at the end.

Measured on trn2 (8 cores, whole kernel): 161.3 us, rel err 3.9e-4.
Known remaining headroom: the PE HAM clock gate spends roughly half the
kernel at 1.2 GHz - a single N=512 matmul stream shows only ~50% activity
at K=4/8 so it cannot re-warm itself; only concurrently-issued row-packed
score pairs push activity over the threshold.  Deeper scores PSUM rings
would fix the exp-lag stalls that trigger re-throttles, but PSUM (8 banks)
is exactly full.  Exploiting E's symmetry (exp only the triangle + DMA
transpose for the mirror) could cut the ACT exp load ~2x if ever needed.
"""

import sys

for _p in ("/opt/trn_rl_repo",):
    if _p not in sys.path:
        sys.path.insert(0, _p)

import numpy as np
import ml_dtypes

T, B, E = 1024, 8, 512
H = 8
HD = E // H  # 64
N_CORES = 8

# ---- tunables -------------------------------------------------------------
MM_DT = "float32r"  # dtype for PE matmuls: "float32r" | "float32" | "bfloat16"
W_DT = "float16"   # dtype of E / W' / v (the attention operands)
GPSIMD_MUL_TILES = 2  # of the 8 s-tiles of each head's W' multiply, how many go to GPSIMD
# ---------------------------------------------------------------------------

# global constant subtracted inside exp (cancels in normalization; keeps
# exp() outputs inside fp16 range: scores/8 - 10 is in [-16, ~6])
EXP_SHIFT = -10.0

_cache = {}


def _build_nc():
    import concourse.bass as bass
    import concourse.tile as tile
    import concourse.mybir as mybir
    from concourse import bacc
    from contextlib import ExitStack

    f32 = mybir.dt.float32
    bf16 = mybir.dt.bfloat16
    mm_dt = getattr(mybir.dt, MM_DT)
    w_dt = getattr(mybir.dt, W_DT)
    Exp = mybir.ActivationFunctionType.Exp

    nc = bacc.Bacc("TRN2", debug=False)

    # DRAM I/O (per-core contents supplied via in_maps)
    xT_d = nc.dram_tensor("xT", [E, T], mm_dt, kind="ExternalInput").ap()
    peT_d = nc.dram_tensor("peT", [T, T], w_dt, kind="ExternalInput").ap()
    wqT_d = nc.dram_tensor("wqT", [E, E], mm_dt, kind="ExternalInput").ap()
    wvT_d = nc.dram_tensor("wvT", [E, E], mm_dt, kind="ExternalInput").ap()
    woT_d = nc.dram_tensor("woT", [E, E], mm_dt, kind="ExternalInput").ap()
    bq_d = nc.dram_tensor("bq", [E], f32, kind="ExternalInput").ap()
    bo2_d = nc.dram_tensor("bo2", [E], mm_dt, kind="ExternalInput").ap()
    ones_d = nc.dram_tensor("ones1", [128], mm_dt, kind="ExternalInput").ap()
    out_d = nc.dram_tensor("out", [T, E], f32, kind="ExternalOutput").ap()

    KT = E // 128   # 4 contraction tiles for the projections
    TT = T // 128   # 8 t-tiles
    NH = T // 512   # 2 psum-bank halves of the t dimension

    def mm(ap):
        return ap

    with ExitStack() as ctx:
        tc = ctx.enter_context(tile.TileContext(nc))

        sing = ctx.enter_context(tc.tile_pool(name="sing", bufs=1))
        p_in = ctx.enter_context(tc.tile_pool(name="p_in", bufs=1))
        p_qv = ctx.enter_context(tc.tile_pool(name="p_qv", bufs=1))
        p_E = ctx.enter_context(tc.tile_pool(name="p_E", bufs=10))
        p_W = ctx.enter_context(tc.tile_pool(name="p_W", bufs=20))
        p_rc = ctx.enter_context(tc.tile_pool(name="p_rc", bufs=4))
        p_rm = ctx.enter_context(tc.tile_pool(name="p_rm", bufs=2))
        p_st = ctx.enter_context(tc.tile_pool(name="p_st", bufs=2))
        p_dr = ctx.enter_context(tc.tile_pool(name="p_dr", bufs=4, space="DRAM"))
        ps_a = ctx.enter_context(tc.tile_pool(name="ps_a", bufs=2, space="PSUM"))
        ps_b = ctx.enter_context(tc.tile_pool(name="ps_b", bufs=4, space="PSUM"))

        # ---- constants / weights into SBUF --------------------------------
        # DMA order matters: the first projection matmuls need xT + wq only,
        # so issue those first; wv arrives for the interleaved v-projection,
        # peT for the first pe-multiplies, and wo (out-proj) last.
        wq_sb = [sing.tile([128, E], mm_dt, tag=f"wq{k}", name="wq") for k in range(KT)]
        wv_sb = [sing.tile([128, E], mm_dt, tag=f"wv{k}", name="wv") for k in range(KT)]
        wo_sb = [sing.tile([128, E], mm_dt, tag=f"wo{k}", name="wo") for k in range(KT)]
        bq_sb = [sing.tile([128, 1], f32, tag=f"bq{k}", name="bq") for k in range(KT)]
        xT_sb = [p_in.tile([128, T], mm_dt, tag=f"xT{k}", name="xT") for k in range(KT)]
        peT_sb = [p_in.tile([128, T], w_dt, tag=f"peT{k}", name="peT") for k in range(TT)]
        for k in range(KT):
            nc.sync.dma_start(out=xT_sb[k], in_=xT_d[k * 128:(k + 1) * 128, :])
            nc.sync.dma_start(out=wq_sb[k], in_=wqT_d[k * 128:(k + 1) * 128, :])
            nc.sync.dma_start(out=bq_sb[k], in_=bq_d[k * 128:(k + 1) * 128].rearrange("(p one) -> p one", one=1))
        ebias = sing.tile([128, 1], f32, tag="ebias")
        nc.vector.memset(ebias, EXP_SHIFT)
        for k in range(KT):
            nc.sync.dma_start(out=wv_sb[k], in_=wvT_d[k * 128:(k + 1) * 128, :])
        for i in range(TT):
            nc.sync.dma_start(out=peT_sb[i], in_=peT_d[i * 128:(i + 1) * 128, :])
        ones1 = sing.tile([1, 128], mm_dt, tag="ones1")
        nc.sync.dma_start(out=ones1, in_=ones_d.unsqueeze(0))
        bo2_sb = sing.tile([1, E], mm_dt, tag="bo2")
        nc.sync.dma_start(out=bo2_sb, in_=bo2_d.unsqueeze(0))
        for k in range(KT):
            nc.sync.dma_start(out=wo_sb[k], in_=woT_d[k * 128:(k + 1) * 128, :])

        # ---- P1: projections ----------------------------------------------
        # qT[e_out, t] with e_out on partitions (4 tiles); includes q-bias.
        qT_sb = [p_qv.tile([128, T], w_dt, tag=f"qT{k}", name="qT") for k in range(KT)]
        # v[t, e_out] natural, with a ones column appended per head:
        # layout (128, 8*65): head h occupies cols [65h, 65h+64), ones at 65h+64.
        v_sb = [p_qv.tile([128, H * (HD + 1)], w_dt, tag=f"v{k}", name="v") for k in range(TT)]

        def emit_qproj(m, nh):
            ps = ps_b.tile([128, 512], f32, tag="slot", name="pp")
            for k in range(KT):
                nc.tensor.matmul(
                    ps, mm(wq_sb[k][:, m * 128:(m + 1) * 128]),
                    mm(xT_sb[k][:, nh * 512:(nh + 1) * 512]),
                    start=(k == 0), stop=(k == KT - 1))
            nc.vector.tensor_scalar_add(
                qT_sb[m][:, nh * 512:(nh + 1) * 512], ps, bq_sb[m])

        def emit_vproj(mt):
            ps = ps_b.tile([128, 512], f32, tag="slot", name="pp")
            for k in range(KT):
                nc.tensor.matmul(
                    ps, mm(xT_sb[k][:, mt * 128:(mt + 1) * 128]), mm(wv_sb[k]),
                    start=(k == 0), stop=(k == KT - 1))
            v_dst = v_sb[mt].rearrange("p (h c) -> p h c", c=HD + 1)
            nc.vector.tensor_copy(
                v_dst[:, :, 0:HD],
                ps.rearrange("p (h c) -> p h c", c=HD))
            nc.vector.memset(v_dst[:, :, HD:HD + 1], 1.0)

        # pair 0's qT upfront; the other 14 projection groups interleave
        # with iteration 0's scores so ACT starts exp'ing ~20us earlier and
        # the packable fp16 score pairs warm the PE clock gate early.
        for nh in range(NH):
            emit_qproj(0, nh)
        proj_rest = [("q", m, nh) for m in range(1, KT) for nh in range(NH)]
        proj_rest += [("v", mt, None) for mt in range(TT)]

        # ---- P2: attention, software-pipelined over head pairs ------------
        # Iteration j produces W' for pair j (scores -> exp -> pe-multiply)
        # while the attention matmuls consume pair j-1's W' tiles.  This
        # keeps the PE stream dense across pair boundaries (HAM stays warm).
        attnT_sb = [p_qv.tile([128, T], mm_dt, tag=f"attnT{k}", name="attnT") for k in range(KT)]
        NP = H // 2  # pairs
        Ws_of = {}   # pair j -> [hh][i] W' tiles

        def emit_scores(j, i):
            # nh-major emission: consecutive MMs alternate row groups
            # (head A rows 0-63, head B rows 64-127) so the PE runs the
            # pair concurrently and overlaps B's LDWEIGHTS with A's MM.
            qt = qT_sb[j]
            scs = [ps_a.tile([128, T], f32, tag="slot", name="sc") for _ in range(2)]
            for nh in range(NH):
                for hh in range(2):
                    r0 = hh * HD
                    nc.tensor.matmul(
                        scs[hh][:, nh * 512:(nh + 1) * 512],
                        mm(qt[r0:r0 + HD, i * 128:(i + 1) * 128]),
                        mm(qt[r0:r0 + HD, nh * 512:(nh + 1) * 512]),
                        start=True, stop=True,
                        tile_position=(r0, 0))
            for hh in range(2):
                Et = p_E.tile([128, T], w_dt, tag="E", name="Et")
                nc.scalar.activation(out=Et, in_=scs[hh], func=Exp, scale=0.125, bias=ebias)
                Wt = p_W.tile([128, T], w_dt, tag="W", name="Wt")
                if i >= TT - GPSIMD_MUL_TILES:
                    nc.gpsimd.tensor_mul(Wt, Et, peT_sb[i])
                else:
                    nc.vector.tensor_mul(Wt, Et, peT_sb[i])
                Ws_of[j][hh][i] = Wt

        def emit_attn_kstep(j, i, at_ps):
            for hh in range(2):
                vcol = 65 * (2 * j + hh)
                for nh in range(NH):
                    nc.tensor.matmul(
                        at_ps[hh][nh],
                        v_sb[i][:, vcol:vcol + HD + 1],
                        Ws_of[j][hh][i][:, nh * 512:(nh + 1) * 512],
                        start=(i == 0), stop=(i == TT - 1))

        def emit_evac(j, at_ps, split_hook=None):
            # plain evacuation first (frees the attention PSUM slots fast);
            # normalization happens afterwards in SBUF, in place.
            for hh in range(2):
                for nh in range(NH):
                    nc.vector.tensor_copy(
                        attnT_sb[j][hh * HD:(hh + 1) * HD,
                                    nh * 512:(nh + 1) * 512],
                        at_ps[hh][nh][0:HD, :])
            rows = p_rc.tile([HD + 1, 4 * 512], f32, tag="rc", name="rows", bufs=2)
            for hh in range(2):
                for nh in range(NH):
                    r = hh * NH + nh
                    nc.vector.tensor_copy(
                        rows[HD:HD + 1, r * 512:(r + 1) * 512],
                        at_ps[hh][nh][HD:HD + 1, :])
            # one low-free-dim reciprocal for the pair's 4 denominator rows
            rg = p_rc.tile([32, 64], f32, tag="rg", name="rg")
            nc.sync.dma_start(
                out=rg,
                in_=rows[HD:HD + 1, :].rearrange("one (a c) -> one a c", c=64))
            rgi = p_rc.tile([32, 64], f32, tag="rgi", name="rgi")
            nc.vector.reciprocal(rgi, rg)
            rrow = p_dr.tile([1, 4 * 512], f32, tag="rrow", name="rrow")
            nc.sync.dma_start(
                out=rrow.rearrange("one (a c) -> one a c", c=64), in_=rgi)
            if split_hook is not None:
                # emit PE-filler work here so its DVE ops sit ahead of the
                # DMA-latency-bound normalization TTs in the DVE FIFO
                split_hook()
            for hh in range(2):
                for nh in range(NH):
                    r = hh * NH + nh
                    rm = p_rm.tile([128, 512], f32, tag="rm", name="rm")
                    rms = rm[hh * HD:(hh + 1) * HD, :]
                    seg = rrow[0:1, r * 512:(r + 1) * 512]
                    bcast = bass.AP(tensor=seg.tensor, offset=seg.offset,
                                    ap=[[0, HD]] + list(seg.ap[1:]))
                    nc.sync.dma_start(out=rms, in_=bcast)
                    dst = attnT_sb[j][hh * HD:(hh + 1) * HD,
                                      nh * 512:(nh + 1) * 512]
                    nc.vector.tensor_mul(dst, dst, rms)

        p3_part = [p_qv.tile([128, E], f32, tag=f"p3p{k}", name="p3p")
                   for k in range(TT)]

        def emit_p3a(mt):
            # out-projection partial over k=0..2 (heads 0..5); evacuated to
            # SBUF so the PSUM slot recycles while pair 3 finishes.
            ps = ps_a.tile([128, 512], f32, tag="slot", name="pp")
            for k in range(KT - 1):
                nc.tensor.matmul(
                    ps, mm(attnT_sb[k][:, mt * 128:(mt + 1) * 128]),
                    mm(wo_sb[k]), start=(k == 0), stop=(k == KT - 2))
            nc.vector.tensor_copy(p3_part[mt], ps)

        # Skewed schedule: at each iteration boundary the first 3 attention
        # k-steps (which need no ACT results) run before the first scores
        # matmul, absorbing the exp-lag so the PE stream never gaps long
        # enough for the HAM clock gate to re-throttle.
        SKEW = 3
        for j in range(NP + 1):
            if j < NP:
                Ws_of[j] = [[None] * TT, [None] * TT]
                if j >= 2:
                    del Ws_of[j - 2]
            cur_at = None
            if j >= 1:
                cur_at = [[ps_b.tile([HD + 1, 512], f32, tag="slot", name="at")
                           for _ in range(NH)] for _ in range(2)]
                for i in range(SKEW):
                    emit_attn_kstep(j - 1, i, cur_at)
            if j == NP:
                # final iteration: finish the attention immediately and start
                # the pair-3 normalization chain; the out-projection k=0..2
                # partials are emitted inside the chain (between the DVE-local
                # half and the DMA-dependent half) so they fill the PE during
                # the reciprocal's DMA round-trip.
                for i in range(SKEW, TT):
                    emit_attn_kstep(j - 1, i, cur_at)

                def _p3a_filler():
                    for mt in range(TT):
                        emit_p3a(mt)
                emit_evac(j - 1, cur_at, split_hook=_p3a_filler)
            else:
                for i in range(TT):
                    emit_scores(j, i)
                    if j == 0:
                        for _ in range(2):
                            if proj_rest:
                                kind, a1, a2 = proj_rest.pop(0)
                                if kind == "q":
                                    emit_qproj(a1, a2)
                                else:
                                    emit_vproj(a1)
                    if j >= 1 and i + SKEW < TT:
                        emit_attn_kstep(j - 1, i + SKEW, cur_at)
                if j >= 1:
                    emit_evac(j - 1, cur_at)

        # ---- P3 tail: k=3 + bias, add the partial, store ------------------
        for mt in range(TT):
            ps = ps_a.tile([128, 512], f32, tag="slot", name="pp")
            nc.tensor.matmul(
                ps, mm(attnT_sb[KT - 1][:, mt * 128:(mt + 1) * 128]),
                mm(wo_sb[KT - 1]), start=True, stop=False)
            nc.tensor.matmul(ps, ones1, bo2_sb, start=False, stop=True)
            st = p_st.tile([128, E], f32, tag="st", name="st")
            nc.vector.tensor_add(st, ps, p3_part[mt])
            nc.sync.dma_start(out=out_d[mt * 128:(mt + 1) * 128, :], in_=st)

    nc.compile()
    return nc


def get_nc():
    if "nc" not in _cache:
        _cache["nc"] = _build_nc()
    return _cache["nc"]


def prep_inputs(query, pe, in_proj_weight, in_proj_bias, out_proj_weight,
                out_proj_bias):
    """Host-side sharding/layout prep. Returns per-core input maps."""
    query = np.asarray(query, dtype=np.float32)
    pe = np.asarray(pe, dtype=np.float32)
    in_proj_weight = np.asarray(in_proj_weight, dtype=np.float32)
    in_proj_bias = np.asarray(in_proj_bias, dtype=np.float32)
    out_proj_weight = np.asarray(out_proj_weight, dtype=np.float32)
    out_proj_bias = np.asarray(out_proj_bias, dtype=np.float32)

    def r32(x):
        # round-to-nearest-even onto e8m11 (fp32r keeps the top 20 bits)
        if MM_DT != "float32r":
            return np.ascontiguousarray(x, dtype=np.float32)
        u = np.ascontiguousarray(x, dtype=np.float32).view(np.uint32)
        u = (u + 0x7FF + ((u >> 12) & 1)) & np.uint32(0xFFFFF000)
        return u.view(np.float32)

    wqT = r32(in_proj_weight[0:E].T)                           # (E, E)
    wvT = r32(in_proj_weight[2 * E:3 * E].T)                   # (E, E)
    woT = r32(out_proj_weight.T)                               # (E, E)
    bq = np.ascontiguousarray(in_proj_bias[0:E])
    bv = in_proj_bias[2 * E:3 * E]
    bo2 = r32(out_proj_weight @ bv + out_proj_bias)

    in_maps = []
    for b in range(N_CORES):
        xT = r32(query[:, b, :].T)                             # (E, T)
        peT = np.ascontiguousarray(pe[b].T).astype(np.float16 if W_DT == "float16" else ml_dtypes.bfloat16)
        in_maps.append({
            "xT": xT, "peT": peT, "wqT": wqT, "wvT": wvT, "woT": woT,
            "bq": bq, "bo2": bo2, "ones1": np.ones(128, dtype=np.float32),
        })
    return in_maps


def kernel(query, pe, in_proj_weight, in_proj_bias, out_proj_weight,
           out_proj_bias):
    from concourse.bass_utils import run_bass_kernel_spmd

    nc = get_nc()
    in_maps = prep_inputs(query, pe, in_proj_weight, in_proj_bias,
                          out_proj_weight, out_proj_bias)
    res = run_bass_kernel_spmd(nc, in_maps, list(range(N_CORES)))
    out = np.empty((T, B, E), dtype=np.float32)
    for b in range(N_CORES):
        out[:, b, :] = res.results[b]["out"]
    return out
